# revision 14
# baseline (speedup 1.0000x reference)
"""Trainium2 Bass kernel for a 2-layer GCN with data-aware attention gate.

Math (per reference):
    src,dst = edges + self-loops; deg = bincount(dst); dinv = rsqrt(deg)
    norm = dinv[src]*dinv[dst]
    h1 = relu(segsum(norm * (x@W1)[src], dst) + b1)
    h2 = relu(segsum(norm * (h1@W2)[src], dst) + b2)
    out = h2 * sigmoid(h2@attn_w + attn_b)

Device strategy (8 NeuronCores, node/dst-sharded):
    norm factorizes: agg[d] = dinv[d] * sum_{e->d} (dinv[s] * T[s]).
    Per layer: each core computes T' = (dinv .* H) @ W for its node shard
    (node-major via one PE transpose), AllGather of T' shards, then per-edge
    dma_gather of T' rows from HBM and PE one-hot selection-matrix matmuls
    accumulate 128-slot window segment sums in PSUM (one bank per window,
    windows strictly sequential so accumulation groups never interleave).
    Host-side prep deals nodes into windows (LPT on degree) so window loads
    are equal, and pads per-(window,parity) edge counts to the same
    128-multiple across cores so the single SPMD instruction stream is valid
    for all 8 cores.  int16 gather indices only reach 32767, so rows are
    gathered from a [TOT/2, 128]-strided view of the [TOT, 64] table
    (elem_step=128) with edges split by source-position parity.
"""

import sys

import numpy as np

_CONC = "/opt/trn_rl_repo"
if _CONC not in sys.path:
    sys.path.insert(0, _CONC)

# ---------------------------------------------------------------------------
# configuration
# ---------------------------------------------------------------------------


class Cfg:
    def __init__(self, N=50000, DIN=128, DH=64, DOUT=32, NC=8, WPC=49, WPG=7):
        self.N, self.DIN, self.DH, self.DOUT = N, DIN, DH, DOUT
        self.NC, self.WPC, self.WPG = NC, WPC, WPG
        assert WPC % WPG == 0
        self.G = WPC // WPG            # gather groups per core
        self.NPC = WPC * 128           # slots per core
        self.TOT = NC * self.NPC       # total slots
        assert self.TOT // 2 <= 32768
        assert DH * 4 == 256           # gather elem must be 256B
        assert self.N <= self.TOT - 2


FULL = Cfg()

# ---------------------------------------------------------------------------
# host-side graph prep
# ---------------------------------------------------------------------------


def _assign_slots(deg, cfg):
    """LPT-deal nodes into NC*WPC bins of <=128 slots, balancing edge load.
    Returns pos[node] -> global slot position."""
    import heapq

    nbins = cfg.NC * cfg.WPC
    cap = np.full(nbins, 128, np.int64)
    order = np.argsort(-deg, kind="stable")
    heap = [(0, b) for b in range(nbins)]
    heapq.heapify(heap)
    count = np.zeros(nbins, np.int64)
    pos = np.empty(cfg.N, np.int64)
    for n in order:
        load, b = heapq.heappop(heap)
        pos[n] = b * 128 + count[b]
        count[b] += 1
        if count[b] < cap[b]:
            heapq.heappush(heap, (load + int(deg[n]), b))
    return pos


def prep(x, edge_index, cfg):
    """Build per-core input arrays and the static (SPMD-uniform) chunk plan."""
    N, NC, WPC, WPG, G = cfg.N, cfg.NC, cfg.WPC, cfg.WPG, cfg.G
    NPC, DIN = cfg.NPC, cfg.DIN

    loops = np.arange(N, dtype=np.int64)
    src = np.concatenate([edge_index[0].astype(np.int64), loops])
    dst = np.concatenate([edge_index[1].astype(np.int64), loops])
    deg = np.bincount(dst, minlength=N).astype(np.float32)
    dinv = (1.0 / np.sqrt(np.maximum(deg, 1e-12))).astype(np.float32)

    pos = _assign_slots(deg, cfg)

    # per-core x shard + per-slot dinv
    x_sh = np.zeros((NC, NPC, DIN), np.float32)
    dinv_slot = np.ones((NC, 128, WPC), np.float32)
    node_of = np.full(cfg.TOT, -1, np.int64)
    node_of[pos] = np.arange(N)
    for c in range(NC):
        seg = node_of[c * NPC:(c + 1) * NPC]
        m = seg >= 0
        x_sh[c][m] = np.asarray(x)[seg[m]]
        dv = np.ones(NPC, np.float32)
        dv[m] = dinv[seg[m]]
        dinv_slot[c] = dv.reshape(WPC, 128).T

    # edge records
    s_pos = pos[src]
    d_pos = pos[dst]
    c_e = d_pos // NPC
    w_e = (d_pos % NPC) // 128          # window within core
    dval_e = (d_pos % 128).astype(np.float32)  # slot within window
    half_e = (s_pos & 1).astype(np.int64)
    gidx_e = (s_pos >> 1).astype(np.int64)

    # bucket edges by (core, window, half)
    from collections import defaultdict
    buckets = defaultdict(list)
    key_all = (c_e * WPC + w_e) * 2 + half_e
    order_e = np.argsort(key_all, kind="stable")
    ks = key_all[order_e]
    bounds = np.searchsorted(ks, np.arange(NC * WPC * 2 + 1))
    for key in range(NC * WPC * 2):
        lo, hi = bounds[key], bounds[key + 1]
        if hi > lo:
            buckets[key] = order_e[lo:hi]

    # per-(window,half) 128-aligned target, equalized across cores
    tgt = np.zeros((WPC, 2), np.int64)
    for w in range(WPC):
        for h in range(2):
            mx = max(len(buckets.get((c * WPC + w) * 2 + h, ()))
                     for c in range(NC))
            tgt[w, h] = int(np.ceil(max(mx, 1) / 128) * 128)

    # per-(group,half) gather segment = concat of member windows' segments
    seglen = np.zeros((G, 2), np.int64)
    for g in range(G):
        for h in range(2):
            seglen[g, h] = tgt[g * WPG:(g + 1) * WPG, h].sum()

    idx_cols = int(sum(seglen[g, h] // 16 for g in range(G) for h in range(2)))
    chunk_tot = int(sum(seglen[g, h] // 128 for g in range(G) for h in range(2)))
    idx_all = np.zeros((NC, 128, idx_cols), np.int16)
    dval_all = np.full((NC, 128, chunk_tot), -1.0, np.float32)

    ioff, coff = {}, {}
    io = co = 0
    for g in range(G):
        for h in range(2):
            ioff[(g, h)] = io
            coff[(g, h)] = co
            io += int(seglen[g, h]) // 16
            co += int(seglen[g, h]) // 128
    # chunk column (within dval_all / gather tile) of window w's half-h run
    wcol = np.zeros((WPC, 2), np.int64)
    for g in range(G):
        for h in range(2):
            c0 = coff[(g, h)]
            for wl in range(WPG):
                w = g * WPG + wl
                wcol[w, h] = c0
                c0 += tgt[w, h] // 128

    for c in range(NC):
        for g in range(G):
            for h in range(2):
                n = int(seglen[g, h])
                gi = np.zeros(n, np.int64)
                dv = np.full(n, -1.0, np.float32)
                p = 0
                for wl in range(WPG):
                    w = g * WPG + wl
                    es = buckets.get((c * WPC + w) * 2 + h, ())
                    ne = len(es)
                    gi[p:p + ne] = gidx_e[es]
                    dv[p:p + ne] = dval_e[es]
                    p += int(tgt[w, h])
                wrapped = gi.reshape(n // 16, 16).T.astype(np.int16)
                idx_all[c, :, ioff[(g, h)]:ioff[(g, h)] + n // 16] = np.tile(
                    wrapped, (8, 1))
                dval_all[c, :, coff[(g, h)]:coff[(g, h)] + n // 128] = (
                    dv.reshape(n // 128, 128).T)

    plan = dict(tgt=tgt, seglen=seglen, ioff=ioff, coff=coff, wcol=wcol,
                idx_cols=idx_cols, chunk_tot=chunk_tot)
    host = dict(x_sh=x_sh, dinv_slot=dinv_slot, idx_all=idx_all,
                dval_all=dval_all, pos=pos)
    return plan, host


# ---------------------------------------------------------------------------
# device kernel
# ---------------------------------------------------------------------------


def build(cfg, plan):
    import os
    import concourse.bass as bass
    import concourse.mybir as mybir
    import concourse.tile as tile
    from concourse import bacc

    STAGE = int(os.environ.get("KERNEL_STAGE", "5"))
    NOGATHER = os.environ.get("KERNEL_NOGATHER", "0") == "1"
    NOCOLL = os.environ.get("KERNEL_NOCOLL", "0") == "1"

    NC, WPC, WPG, G = cfg.NC, cfg.WPC, cfg.WPG, cfg.G
    NPC, TOT, DIN, DH, DOUT = cfg.NPC, cfg.TOT, cfg.DIN, cfg.DH, cfg.DOUT
    f32 = mybir.dt.float32
    tgt, seglen = plan["tgt"], plan["seglen"]
    ioff, coff, wcol = plan["ioff"], plan["coff"], plan["wcol"]
    AF = mybir.ActivationFunctionType

    nc = bacc.Bacc(
        "TRN2", target_bir_lowering=False, debug=False,
        num_devices=NC, num_swdge_queues=2,
    )

    # I/O
    x_d = nc.dram_tensor("x_sh", [NPC, DIN], f32, kind="ExternalInput")
    w1_d = nc.dram_tensor("w1", [DIN, DH], f32, kind="ExternalInput")
    w2_d = nc.dram_tensor("w2", [DH, DOUT], f32, kind="ExternalInput")
    b1_d = nc.dram_tensor("b1rep", [128, DH], f32, kind="ExternalInput")
    b2_d = nc.dram_tensor("b2rep", [128, DOUT], f32, kind="ExternalInput")
    aw_d = nc.dram_tensor("awrep", [128, DOUT], f32, kind="ExternalInput")
    ab_d = nc.dram_tensor("abcol", [128, 1], f32, kind="ExternalInput")
    dv_d = nc.dram_tensor("dinv_slot", [128, WPC], f32, kind="ExternalInput")
    id_d = nc.dram_tensor("ident", [128, 128], f32, kind="ExternalInput")
    gi_d = nc.dram_tensor("giota", [128, 128], f32, kind="ExternalInput")
    ix_d = nc.dram_tensor("idx_all", [128, plan["idx_cols"]], mybir.dt.int16,
                          kind="ExternalInput")
    dvl_d = nc.dram_tensor("dval_all", [128, plan["chunk_tot"]], f32,
                           kind="ExternalInput")
    out_d = nc.dram_tensor("out_sh", [NPC, DOUT], f32, kind="ExternalOutput")

    rg = [list(range(NC))]

    with tile.TileContext(nc) as tc:
        with tc.tile_pool(name="const", bufs=1) as cpool:
            def load(dram, shape, dt=f32):
                t = cpool.tile(shape, dt, tag=dram.name, name=dram.name + "_s")
                nc.sync.dma_start(t[:], dram.ap())
                return t

            w1_s = load(w1_d, [DIN, DH])
            w2_s = load(w2_d, [DH, DOUT])
            b1_s = load(b1_d, [128, DH])
            b2_s = load(b2_d, [128, DOUT])
            aw_s = load(aw_d, [128, DOUT])
            ab_s = load(ab_d, [128, 1])
            dv_s = load(dv_d, [128, WPC])
            id_s = load(id_d, [128, 128])
            gi_s = load(gi_d, [128, 128])
            ix_s = load(ix_d, [128, plan["idx_cols"]], mybir.dt.int16)
            dvl_s = load(dvl_d, [128, plan["chunk_tot"]])

            with tc.tile_pool(name="dram", bufs=1, space="DRAM") as dpool:
                t1_shard = dpool.tile([NPC, DH], f32, tag="t1s", name="t1s")
                t1_full = dpool.tile([TOT, DH], f32, tag="t1f", name="t1f",
                                     addr_space="Shared")
                t2_shard = dpool.tile([NPC, DH], f32, tag="t2s", name="t2s")
                t2_full = dpool.tile([TOT, DH], f32, tag="t2f", name="t2f",
                                     addr_space="Shared")

                # ---- phase 1: T1' = (dinv .* x) @ W1, node-major
                with (
                    tc.tile_pool(name="tf_in", bufs=3) as pin,
                    tc.tile_pool(name="tf_ps", bufs=2, space="PSUM") as pps,
                    tc.tile_pool(name="tf_sb", bufs=3) as psb,
                ):
                    for w in range(WPC):
                        xt = pin.tile([128, DIN], f32, tag="xt", name="xt")
                        nc.sync.dma_start(
                            xt[:], x_d.ap()[w * 128:(w + 1) * 128, :])
                        xs = psb.tile([128, DIN], f32, tag="xs", name="xs")
                        nc.vector.tensor_scalar_mul(
                            xs[:], xt[:], dv_s[:, w:w + 1])
                        xtp = pps.tile([128, DIN], f32, tag="xtp", name="xtp")
                        nc.tensor.transpose(xtp[:], xs[:], id_s[:])
                        xts = psb.tile([128, DIN], f32, tag="xts", name="xts")
                        nc.vector.tensor_copy(xts[:], xtp[:])
                        hp = pps.tile([128, DH], f32, tag="hp", name="hp")
                        nc.tensor.matmul(hp[:], lhsT=xts[:], rhs=w1_s[:],
                                         start=True, stop=True)
                        hs = psb.tile([128, DH], f32, tag="hs", name="hs")
                        nc.vector.tensor_copy(hs[:], hp[:])
                        nc.sync.dma_start(
                            t1_shard[w * 128:(w + 1) * 128, :], hs[:])

                if STAGE == 1:
                    # debug: dump T1' shard columns into out and stop
                    with tc.tile_pool(name="dbg", bufs=2) as dbg:
                        for w in range(WPC):
                            d = dbg.tile([128, DOUT], f32, name="d")
                            nc.sync.dma_start(
                                d[:],
                                t1_shard[w * 128:(w + 1) * 128, :DOUT])
                            d2 = dbg.tile([128, DOUT], f32, name="d2")
                            nc.vector.tensor_copy(d2[:], d[:])
                            nc.sync.dma_start(
                                out_d.ap()[w * 128:(w + 1) * 128, :], d2[:])
                    nc.compile()
                    return nc

                # ---- phase 2: AllGather layer-1 table
                if NOCOLL:
                    nc.sync.dma_start(t1_full[0:NPC, :], t1_shard[:])
                else:
                    nc.gpsimd.collective_compute(
                        "AllGather", mybir.AluOpType.bypass, replica_groups=rg,
                        ins=[t1_shard[:]], outs=[t1_full[:]],
                    )

                if STAGE == 2:
                    with tc.tile_pool(name="dbg", bufs=2) as dbg:
                        for w in range(WPC):
                            d = dbg.tile([128, DOUT], f32, name="d")
                            nc.sync.dma_start(
                                d[:], t1_full[w * 128:(w + 1) * 128, :DOUT])
                            d2 = dbg.tile([128, DOUT], f32, name="d2")
                            nc.vector.tensor_copy(d2[:], d[:])
                            nc.sync.dma_start(
                                out_d.ap()[w * 128:(w + 1) * 128, :], d2[:])
                    nc.compile()
                    return nc

                # ---- aggregation: gather rows + one-hot matmul segment sums
                def aggregate(full, ncols, flush_fn):
                    fv = full.rearrange("(a b) d -> a (b d)", b=2)
                    with (
                        tc.tile_pool(name="gpool", bufs=2) as gp,
                        tc.tile_pool(name="spool", bufs=4) as sp,
                        tc.tile_pool(name="apsum", bufs=4, space="PSUM") as aps,
                    ):
                        for g in range(G):
                            gts = {}
                            for h in range(2):
                                n = int(seglen[g, h])
                                nch = n // 128
                                gt = gp.tile([128, nch * DH], f32,
                                             tag=f"g{h}", name=f"gt{h}")
                                io = ioff[(g, h)]
                                if NOGATHER:
                                    nc.sync.dma_start(
                                        gt[:],
                                        full[0:128 * nch, :].rearrange(
                                            "(p c) d -> p (c d)", p=128))
                                else:
                                    nc.gpsimd.dma_gather(
                                        out_ap=gt[:].rearrange(
                                            "p (c d) -> p c d", d=DH),
                                        in_ap=fv[:, h * DH:(h + 1) * DH],
                                        idxs_ap=ix_s[:, io:io + n // 16],
                                        num_idxs=n, num_idxs_reg=n,
                                        elem_size=DH, elem_step=2 * DH,
                                        queue_num=0, single_packet=False,
                                    )
                                gts[h] = gt
                            for wl in range(WPG):
                                w = g * WPG + wl
                                ps = aps.tile([128, DH], f32, tag="agg",
                                              name="agg")
                                chunks = (
                                    [(0, k) for k in range(int(tgt[w, 0]) // 128)]
                                    + [(1, k) for k in range(int(tgt[w, 1]) // 128)]
                                )
                                for j, (h, k) in enumerate(chunks):
                                    tcol = int(wcol[w, h] - coff[(g, h)]) + k
                                    dcol = int(wcol[w, h]) + k
                                    S = sp.tile([128, 128], f32, tag="S",
                                                name="S")
                                    nc.vector.tensor_tensor(
                                        out=S[:],
                                        in0=dvl_s[:, dcol:dcol + 1]
                                        .to_broadcast([128, 128]),
                                        in1=gi_s[:],
                                        op=mybir.AluOpType.is_equal,
                                    )
                                    nc.tensor.matmul(
                                        ps[:, :ncols],
                                        lhsT=S[:],
                                        rhs=gts[h][:, tcol * DH:tcol * DH + ncols],
                                        start=(j == 0),
                                        stop=(j == len(chunks) - 1),
                                    )
                                flush_fn(w, ps[:, :ncols])

                if STAGE == 3:
                    with tc.tile_pool(name="dbg", bufs=2) as dbg:
                        def flush_dbg(w, agg):
                            d2 = dbg.tile([128, DOUT], f32, name="d2")
                            nc.vector.tensor_copy(d2[:], agg[:, :DOUT])
                            nc.sync.dma_start(
                                out_d.ap()[w * 128:(w + 1) * 128, :], d2[:])
                        aggregate(t1_full[:], DH, flush_dbg)
                    nc.compile()
                    return nc

                # ---- layer-1 flush: h=relu(dinv*agg+b1); T2'=(dinv.*h)@W2
                with (
                    tc.tile_pool(name="fl_sb", bufs=3) as fsb,
                    tc.tile_pool(name="fl_ps", bufs=2, space="PSUM") as fps,
                ):
                    def flush1(w, agg):
                        v = fsb.tile([128, DH], f32, tag="v", name="v")
                        nc.vector.tensor_scalar_mul(
                            v[:], agg, dv_s[:, w:w + 1])
                        v2 = fsb.tile([128, DH], f32, tag="v2", name="v2")
                        nc.vector.tensor_add(out=v2[:], in0=v[:], in1=b1_s[:])
                        h2 = fsb.tile([128, DH], f32, tag="h2", name="h2")
                        # dinv*relu(x) == relu(dinv*x) since dinv>0
                        nc.scalar.activation(h2[:], v2[:], func=AF.Relu,
                                             scale=dv_s[:, w:w + 1])
                        htp = fps.tile([DH, 128], f32, tag="htp", name="htp")
                        nc.tensor.transpose(htp[:], h2[:], id_s[:])
                        hts = fsb.tile([DH, 128], f32, tag="hts", name="hts")
                        nc.vector.tensor_copy(hts[:], htp[:])
                        t2p = fps.tile([128, DOUT], f32, tag="t2p", name="t2p")
                        nc.tensor.matmul(t2p[:], lhsT=hts[:], rhs=w2_s[:],
                                         start=True, stop=True)
                        t2sb = fsb.tile([128, DH], f32, tag="t2sb",
                                        name="t2sb")
                        nc.vector.memset(t2sb[:, DOUT:], 0.0)
                        nc.vector.tensor_copy(t2sb[:, :DOUT], t2p[:])
                        nc.sync.dma_start(
                            t2_shard[w * 128:(w + 1) * 128, :], t2sb[:])

                    aggregate(t1_full[:], DH, flush1)

                    # ---- phase 4: AllGather layer-2 table
                    nc.gpsimd.collective_compute(
                        "AllGather", mybir.AluOpType.bypass, replica_groups=rg,
                        ins=[t2_shard[:]], outs=[t2_full[:]],
                    )

                    if STAGE == 4:
                        with tc.tile_pool(name="dbg", bufs=2) as dbg:
                            for w in range(WPC):
                                d = dbg.tile([128, DOUT], f32, name="d")
                                nc.sync.dma_start(
                                    d[:],
                                    t2_full[w * 128:(w + 1) * 128, :DOUT])
                                d2 = dbg.tile([128, DOUT], f32, name="d2")
                                nc.vector.tensor_copy(d2[:], d[:])
                                nc.sync.dma_start(
                                    out_d.ap()[w * 128:(w + 1) * 128, :],
                                    d2[:])
                        nc.compile()
                        return nc

                    # ---- layer-2 flush: h2 + attention gate -> out
                    def flush2(w, agg):
                        v = fsb.tile([128, DOUT], f32, tag="f2v", name="f2v")
                        nc.vector.tensor_scalar_mul(
                            v[:], agg, dv_s[:, w:w + 1])
                        v2 = fsb.tile([128, DOUT], f32, tag="f2v2",
                                      name="f2v2")
                        nc.vector.tensor_add(out=v2[:], in0=v[:], in1=b2_s[:])
                        hh = fsb.tile([128, DOUT], f32, tag="f2h", name="f2h")
                        nc.scalar.activation(hh[:], v2[:], func=AF.Relu)
                        a = fsb.tile([128, DOUT], f32, tag="f2a", name="f2a")
                        nc.vector.tensor_mul(out=a[:], in0=hh[:], in1=aw_s[:])
                        ar = fsb.tile([128, 1], f32, tag="f2ar", name="f2ar")
                        nc.vector.tensor_reduce(
                            ar[:], a[:], axis=mybir.AxisListType.X,
                            op=mybir.AluOpType.add)
                        at = fsb.tile([128, 1], f32, tag="f2at", name="f2at")
                        nc.scalar.activation(at[:], ar[:], func=AF.Sigmoid,
                                             bias=ab_s[:, :1])
                        o = fsb.tile([128, DOUT], f32, tag="f2o", name="f2o")
                        nc.vector.tensor_scalar_mul(o[:], hh[:], at[:])
                        nc.sync.dma_start(
                            out_d.ap()[w * 128:(w + 1) * 128, :], o[:])

                    aggregate(t2_full[:], DOUT, flush2)

    nc.compile()
    return nc


# ---------------------------------------------------------------------------
# entry point
# ---------------------------------------------------------------------------


def _make_in_maps(cfg, host, W1, b1, W2, b2, attn_w, attn_b):
    NC = cfg.NC
    ident = np.eye(128, dtype=np.float32)
    giota = np.tile(np.arange(128, dtype=np.float32), (128, 1))
    in_maps = []
    for c in range(NC):
        in_maps.append({
            "x_sh": host["x_sh"][c],
            "w1": np.asarray(W1, np.float32),
            "w2": np.asarray(W2, np.float32),
            "b1rep": np.tile(np.asarray(b1, np.float32), (128, 1)),
            "b2rep": np.tile(np.asarray(b2, np.float32), (128, 1)),
            "awrep": np.tile(np.asarray(attn_w, np.float32).reshape(1, -1),
                             (128, 1)),
            "abcol": np.full((128, 1),
                             np.asarray(attn_b, np.float32).reshape(-1)[0],
                             np.float32),
            "dinv_slot": host["dinv_slot"][c],
            "ident": ident,
            "giota": giota,
            "idx_all": host["idx_all"][c],
            "dval_all": host["dval_all"][c],
        })
    return in_maps


def run(x, edge_index, W1, b1, W2, b2, attn_w, attn_b, cfg=None,
        backend="hw", trace=False):
    cfg = cfg or FULL
    plan, host = prep(x, edge_index, cfg)
    nc = build(cfg, plan)
    in_maps = _make_in_maps(cfg, host, W1, b1, W2, b2, attn_w, attn_b)

    if backend == "sim":
        from concourse.bass_interp import MultiCoreSim
        sim = MultiCoreSim(nc, num_cores=cfg.NC, trace=False)
        for c, core in enumerate(sim.cores.values()):
            for name, arr in in_maps[c].items():
                core.tensor(name)[:] = arr
        sim.simulate()
        outs = [core.tensor("out_sh").copy() for core in sim.cores.values()]
        exec_ns = None
    else:
        from concourse import bass_utils
        from concourse.bass_interp import get_hw_module
        old = nc.m
        nc.m = get_hw_module(nc.m)
        try:
            res = bass_utils.run_bass_kernel_spmd(
                nc, in_maps, core_ids=list(range(cfg.NC)), trace=trace)
        finally:
            nc.m = old
        outs = [res.results[c]["out_sh"] for c in range(cfg.NC)]
        exec_ns = res.exec_time_ns

    full = np.concatenate(outs, axis=0)  # [TOT, DOUT] in slot order
    out = full[host["pos"]]              # unpermute -> [N, DOUT]
    return np.ascontiguousarray(out), exec_ns


def kernel(x, edge_index, W1, b1, W2, b2, attn_w, attn_b):
    out, _ = run(x, edge_index, W1, b1, W2, b2, attn_w, attn_b,
                 cfg=FULL, backend="hw", trace=False)
    return out


# revision 18
# speedup vs baseline: 1.1895x; 1.1895x over previous
"""Trainium2 Bass kernel for a 2-layer GCN with data-aware attention gate.

Math (per reference):
    src,dst = edges + self-loops; deg = bincount(dst); dinv = rsqrt(deg)
    norm = dinv[src]*dinv[dst]
    h1 = relu(segsum(norm * (x@W1)[src], dst) + b1)
    h2 = relu(segsum(norm * (h1@W2)[src], dst) + b2)
    out = h2 * sigmoid(h2@attn_w + attn_b)

Device strategy (8 NeuronCores, node/dst-sharded):
    norm factorizes: agg[d] = dinv[d] * sum_{e->d} (dinv[s] * T[s]).
    Per layer: each core computes T' = (dinv .* H) @ W for its node shard
    (node-major via one PE transpose), AllGather of T' shards, then per-edge
    dma_gather of T' rows from HBM and PE one-hot selection-matrix matmuls
    accumulate 128-slot window segment sums in PSUM (one bank per window,
    windows strictly sequential so accumulation groups never interleave).
    Host-side prep deals nodes into windows (LPT on degree) so window loads
    are equal, and pads per-(window,parity) edge counts to the same
    128-multiple across cores so the single SPMD instruction stream is valid
    for all 8 cores.  int16 gather indices only reach 32767, so rows are
    gathered from a [TOT/2, 128]-strided view of the [TOT, 64] table
    (elem_step=128) with edges split by source-position parity.
"""

import sys

import numpy as np

_CONC = "/opt/trn_rl_repo"
if _CONC not in sys.path:
    sys.path.insert(0, _CONC)

# ---------------------------------------------------------------------------
# configuration
# ---------------------------------------------------------------------------


class Cfg:
    def __init__(self, N=50000, DIN=128, DH=64, DOUT=32, NC=8, WPC=49, WPG=7):
        self.N, self.DIN, self.DH, self.DOUT = N, DIN, DH, DOUT
        self.NC, self.WPC, self.WPG = NC, WPC, WPG
        assert WPC % WPG == 0
        self.G = WPC // WPG            # gather groups per core
        self.NPC = WPC * 128           # slots per core
        self.TOT = NC * self.NPC       # total slots
        assert self.TOT // 2 <= 32768
        assert DH * 4 == 256           # gather elem must be 256B
        assert self.N <= self.TOT - 2


FULL = Cfg()

# ---------------------------------------------------------------------------
# host-side graph prep
# ---------------------------------------------------------------------------


def _assign_slots(deg, cfg):
    """LPT-deal nodes into NC*WPC bins of <=128 slots, balancing edge load.
    Returns pos[node] -> global slot position."""
    import heapq

    nbins = cfg.NC * cfg.WPC
    cap = np.full(nbins, 128, np.int64)
    order = np.argsort(-deg, kind="stable")
    heap = [(0, b) for b in range(nbins)]
    heapq.heapify(heap)
    count = np.zeros(nbins, np.int64)
    pos = np.empty(cfg.N, np.int64)
    for n in order:
        load, b = heapq.heappop(heap)
        pos[n] = b * 128 + count[b]
        count[b] += 1
        if count[b] < cap[b]:
            heapq.heappush(heap, (load + int(deg[n]), b))
    return pos


def prep(x, edge_index, cfg):
    """Build per-core input arrays and the static (SPMD-uniform) chunk plan."""
    N, NC, WPC, WPG, G = cfg.N, cfg.NC, cfg.WPC, cfg.WPG, cfg.G
    NPC, DIN = cfg.NPC, cfg.DIN

    loops = np.arange(N, dtype=np.int64)
    src = np.concatenate([edge_index[0].astype(np.int64), loops])
    dst = np.concatenate([edge_index[1].astype(np.int64), loops])
    deg = np.bincount(dst, minlength=N).astype(np.float32)
    dinv = (1.0 / np.sqrt(np.maximum(deg, 1e-12))).astype(np.float32)

    pos = _assign_slots(deg, cfg)

    # per-core x shard + per-slot dinv
    x_sh = np.zeros((NC, NPC, DIN), np.float32)
    dinv_slot = np.ones((NC, 128, WPC), np.float32)
    node_of = np.full(cfg.TOT, -1, np.int64)
    node_of[pos] = np.arange(N)
    for c in range(NC):
        seg = node_of[c * NPC:(c + 1) * NPC]
        m = seg >= 0
        x_sh[c][m] = np.asarray(x)[seg[m]]
        dv = np.ones(NPC, np.float32)
        dv[m] = dinv[seg[m]]
        dinv_slot[c] = dv.reshape(WPC, 128).T

    # edge records
    s_pos = pos[src]
    d_pos = pos[dst]
    c_e = d_pos // NPC
    w_e = (d_pos % NPC) // 128          # window within core
    dval_e = (d_pos % 128).astype(np.float32)  # slot within window
    half_e = (s_pos & 1).astype(np.int64)
    gidx_e = (s_pos >> 1).astype(np.int64)

    # bucket edges by (core, window, half)
    from collections import defaultdict
    buckets = defaultdict(list)
    key_all = (c_e * WPC + w_e) * 2 + half_e
    order_e = np.argsort(key_all, kind="stable")
    ks = key_all[order_e]
    bounds = np.searchsorted(ks, np.arange(NC * WPC * 2 + 1))
    for key in range(NC * WPC * 2):
        lo, hi = bounds[key], bounds[key + 1]
        if hi > lo:
            buckets[key] = order_e[lo:hi]

    # per-(window,half) 128-aligned target, equalized across cores
    tgt = np.zeros((WPC, 2), np.int64)
    for w in range(WPC):
        for h in range(2):
            mx = max(len(buckets.get((c * WPC + w) * 2 + h, ()))
                     for c in range(NC))
            tgt[w, h] = int(np.ceil(max(mx, 1) / 128) * 128)

    # per-(group,half) gather segment = concat of member windows' segments
    seglen = np.zeros((G, 2), np.int64)
    for g in range(G):
        for h in range(2):
            seglen[g, h] = tgt[g * WPG:(g + 1) * WPG, h].sum()

    idx_cols = int(sum(seglen[g, h] // 16 for g in range(G) for h in range(2)))
    chunk_tot = int(sum(seglen[g, h] // 128 for g in range(G) for h in range(2)))
    idx_all = np.zeros((NC, 128, idx_cols), np.int16)
    dval_all = np.full((NC, 128, chunk_tot), -1.0, np.float32)

    ioff, coff = {}, {}
    io = co = 0
    for g in range(G):
        for h in range(2):
            ioff[(g, h)] = io
            coff[(g, h)] = co
            io += int(seglen[g, h]) // 16
            co += int(seglen[g, h]) // 128
    # chunk column (within dval_all / gather tile) of window w's half-h run
    wcol = np.zeros((WPC, 2), np.int64)
    for g in range(G):
        for h in range(2):
            c0 = coff[(g, h)]
            for wl in range(WPG):
                w = g * WPG + wl
                wcol[w, h] = c0
                c0 += tgt[w, h] // 128

    for c in range(NC):
        for g in range(G):
            for h in range(2):
                n = int(seglen[g, h])
                gi = np.zeros(n, np.int64)
                dv = np.full(n, -1.0, np.float32)
                p = 0
                for wl in range(WPG):
                    w = g * WPG + wl
                    es = buckets.get((c * WPC + w) * 2 + h, ())
                    ne = len(es)
                    gi[p:p + ne] = gidx_e[es]
                    dv[p:p + ne] = dval_e[es]
                    p += int(tgt[w, h])
                wrapped = gi.reshape(n // 16, 16).T.astype(np.int16)
                idx_all[c, :, ioff[(g, h)]:ioff[(g, h)] + n // 16] = np.tile(
                    wrapped, (8, 1))
                dval_all[c, :, coff[(g, h)]:coff[(g, h)] + n // 128] = (
                    dv.reshape(n // 128, 128).T)

    plan = dict(tgt=tgt, seglen=seglen, ioff=ioff, coff=coff, wcol=wcol,
                idx_cols=idx_cols, chunk_tot=chunk_tot)
    host = dict(x_sh=x_sh, dinv_slot=dinv_slot, idx_all=idx_all,
                dval_all=dval_all, pos=pos)
    return plan, host


# ---------------------------------------------------------------------------
# device kernel
# ---------------------------------------------------------------------------


def build(cfg, plan):
    import os
    import concourse.bass as bass
    import concourse.mybir as mybir
    import concourse.tile as tile
    from concourse import bacc

    STAGE = int(os.environ.get("KERNEL_STAGE", "5"))
    NOGATHER = os.environ.get("KERNEL_NOGATHER", "0") == "1"
    NOCOLL = os.environ.get("KERNEL_NOCOLL", "0") == "1"

    NC, WPC, WPG, G = cfg.NC, cfg.WPC, cfg.WPG, cfg.G
    NPC, TOT, DIN, DH, DOUT = cfg.NPC, cfg.TOT, cfg.DIN, cfg.DH, cfg.DOUT
    f32 = mybir.dt.float32
    tgt, seglen = plan["tgt"], plan["seglen"]
    ioff, coff, wcol = plan["ioff"], plan["coff"], plan["wcol"]
    AF = mybir.ActivationFunctionType

    nc = bacc.Bacc(
        "TRN2", target_bir_lowering=False, debug=False,
        num_devices=NC, num_swdge_queues=4,
    )

    # I/O
    x_d = nc.dram_tensor("x_sh", [NPC, DIN], f32, kind="ExternalInput")
    w1_d = nc.dram_tensor("w1", [DIN, DH], f32, kind="ExternalInput")
    w2_d = nc.dram_tensor("w2", [DH, DOUT], f32, kind="ExternalInput")
    b1_d = nc.dram_tensor("b1rep", [128, DH], f32, kind="ExternalInput")
    b2_d = nc.dram_tensor("b2rep", [128, DOUT], f32, kind="ExternalInput")
    aw_d = nc.dram_tensor("awrep", [128, DOUT], f32, kind="ExternalInput")
    ab_d = nc.dram_tensor("abcol", [128, 1], f32, kind="ExternalInput")
    dv_d = nc.dram_tensor("dinv_slot", [128, WPC], f32, kind="ExternalInput")
    id_d = nc.dram_tensor("ident", [128, 128], f32, kind="ExternalInput")
    gi_d = nc.dram_tensor("giota", [128, 128], f32, kind="ExternalInput")
    ix_d = nc.dram_tensor("idx_all", [128, plan["idx_cols"]], mybir.dt.int16,
                          kind="ExternalInput")
    dvl_d = nc.dram_tensor("dval_all", [128, plan["chunk_tot"]], f32,
                           kind="ExternalInput")
    out_d = nc.dram_tensor("out_sh", [NPC, DOUT], f32, kind="ExternalOutput")

    rg = [list(range(NC))]

    with tile.TileContext(nc) as tc:
        with tc.tile_pool(name="const", bufs=1) as cpool:
            def load(dram, shape, dt=f32):
                t = cpool.tile(shape, dt, tag=dram.name, name=dram.name + "_s")
                nc.sync.dma_start(t[:], dram.ap())
                return t

            w1_s = load(w1_d, [DIN, DH])
            w2_s = load(w2_d, [DH, DOUT])
            b1_s = load(b1_d, [128, DH])
            b2_s = load(b2_d, [128, DOUT])
            aw_s = load(aw_d, [128, DOUT])
            ab_s = load(ab_d, [128, 1])
            dv_s = load(dv_d, [128, WPC])
            id_s = load(id_d, [128, 128])
            gi_s = load(gi_d, [128, 128])
            ix_s = load(ix_d, [128, plan["idx_cols"]], mybir.dt.int16)
            dvl_s = load(dvl_d, [128, plan["chunk_tot"]])

            with tc.tile_pool(name="dram", bufs=1, space="DRAM") as dpool:
                t1_shard = dpool.tile([NPC, DH], f32, tag="t1s", name="t1s")
                t1_full = dpool.tile([TOT, DH], f32, tag="t1f", name="t1f",
                                     addr_space="Shared")
                t2_shard = dpool.tile([NPC, DH], f32, tag="t2s", name="t2s")
                t2_full = dpool.tile([TOT, DH], f32, tag="t2f", name="t2f",
                                     addr_space="Shared")

                # ---- phase 1: T1' = (dinv .* x) @ W1, node-major
                with (
                    tc.tile_pool(name="tf_in", bufs=3) as pin,
                    tc.tile_pool(name="tf_ps", bufs=2, space="PSUM") as pps,
                    tc.tile_pool(name="tf_sb", bufs=3) as psb,
                ):
                    for w in range(WPC):
                        xt = pin.tile([128, DIN], f32, tag="xt", name="xt")
                        nc.sync.dma_start(
                            xt[:], x_d.ap()[w * 128:(w + 1) * 128, :])
                        xs = psb.tile([128, DIN], f32, tag="xs", name="xs")
                        nc.vector.tensor_scalar_mul(
                            xs[:], xt[:], dv_s[:, w:w + 1])
                        xtp = pps.tile([128, DIN], f32, tag="xtp", name="xtp")
                        nc.tensor.transpose(xtp[:], xs[:], id_s[:])
                        xts = psb.tile([128, DIN], f32, tag="xts", name="xts")
                        nc.vector.tensor_copy(xts[:], xtp[:])
                        hp = pps.tile([128, DH], f32, tag="hp", name="hp")
                        nc.tensor.matmul(hp[:], lhsT=xts[:], rhs=w1_s[:],
                                         start=True, stop=True)
                        hs = psb.tile([128, DH], f32, tag="hs", name="hs")
                        nc.vector.tensor_copy(hs[:], hp[:])
                        nc.sync.dma_start(
                            t1_shard[w * 128:(w + 1) * 128, :], hs[:])

                if STAGE == 1:
                    # debug: dump T1' shard columns into out and stop
                    with tc.tile_pool(name="dbg", bufs=2) as dbg:
                        for w in range(WPC):
                            d = dbg.tile([128, DOUT], f32, name="d")
                            nc.sync.dma_start(
                                d[:],
                                t1_shard[w * 128:(w + 1) * 128, :DOUT])
                            d2 = dbg.tile([128, DOUT], f32, name="d2")
                            nc.vector.tensor_copy(d2[:], d[:])
                            nc.sync.dma_start(
                                out_d.ap()[w * 128:(w + 1) * 128, :], d2[:])
                    nc.compile()
                    return nc

                # ---- phase 2: AllGather layer-1 table
                if NOCOLL:
                    nc.sync.dma_start(t1_full[0:NPC, :], t1_shard[:])
                else:
                    nc.gpsimd.collective_compute(
                        "AllGather", mybir.AluOpType.bypass, replica_groups=rg,
                        ins=[t1_shard[:]], outs=[t1_full[:]],
                    )

                if STAGE == 2:
                    with tc.tile_pool(name="dbg", bufs=2) as dbg:
                        for w in range(WPC):
                            d = dbg.tile([128, DOUT], f32, name="d")
                            nc.sync.dma_start(
                                d[:], t1_full[w * 128:(w + 1) * 128, :DOUT])
                            d2 = dbg.tile([128, DOUT], f32, name="d2")
                            nc.vector.tensor_copy(d2[:], d[:])
                            nc.sync.dma_start(
                                out_d.ap()[w * 128:(w + 1) * 128, :], d2[:])
                    nc.compile()
                    return nc

                # ---- aggregation: gather rows + one-hot matmul segment sums
                def aggregate(full, ncols, flush_fn, qctr=[0]):
                    fv = full.rearrange("(a b) d -> a (b d)", b=2)
                    with (
                        tc.tile_pool(name="gpool", bufs=2) as gp,
                        tc.tile_pool(name="spool", bufs=6) as sp,
                        tc.tile_pool(name="apsum", bufs=4, space="PSUM") as aps,
                    ):
                        for g in range(G):
                            gts = {}
                            for h in range(2):
                                n = int(seglen[g, h])
                                nch = n // 128
                                gt = gp.tile([128, nch * DH], f32,
                                             tag=f"g{h}", name=f"gt{h}")
                                io = ioff[(g, h)]
                                if NOGATHER:
                                    nc.sync.dma_start(
                                        gt[:],
                                        full[0:128 * nch, :].rearrange(
                                            "(p c) d -> p (c d)", p=128))
                                else:
                                    nc.gpsimd.dma_gather(
                                        out_ap=gt[:].rearrange(
                                            "p (c d) -> p c d", d=DH),
                                        in_ap=fv[:, h * DH:(h + 1) * DH],
                                        idxs_ap=ix_s[:, io:io + n // 16],
                                        num_idxs=n, num_idxs_reg=n,
                                        elem_size=DH, elem_step=2 * DH,
                                        queue_num=qctr[0] % 4,
                                        single_packet=False,
                                    )
                                    qctr[0] += 1
                                gts[h] = gt
                            for wl in range(WPG):
                                w = g * WPG + wl
                                ps = aps.tile([128, DH], f32, tag="agg",
                                              name="agg")
                                chunks = (
                                    [(0, k) for k in range(int(tgt[w, 0]) // 128)]
                                    + [(1, k) for k in range(int(tgt[w, 1]) // 128)]
                                )
                                for j, (h, k) in enumerate(chunks):
                                    tcol = int(wcol[w, h] - coff[(g, h)]) + k
                                    dcol = int(wcol[w, h]) + k
                                    S = sp.tile([128, 128], f32, tag="S",
                                                name="S")
                                    nc.vector.tensor_scalar(
                                        out=S[:],
                                        in0=gi_s[:],
                                        scalar1=dvl_s[:, dcol:dcol + 1],
                                        scalar2=None,
                                        op0=mybir.AluOpType.is_equal,
                                    )
                                    nc.tensor.matmul(
                                        ps[:, :ncols],
                                        lhsT=S[:],
                                        rhs=gts[h][:, tcol * DH:tcol * DH + ncols],
                                        start=(j == 0),
                                        stop=(j == len(chunks) - 1),
                                    )
                                flush_fn(w, ps[:, :ncols])

                if STAGE == 3:
                    with tc.tile_pool(name="dbg", bufs=2) as dbg:
                        def flush_dbg(w, agg):
                            d2 = dbg.tile([128, DOUT], f32, name="d2")
                            nc.vector.tensor_copy(d2[:], agg[:, :DOUT])
                            nc.sync.dma_start(
                                out_d.ap()[w * 128:(w + 1) * 128, :], d2[:])
                        aggregate(t1_full[:], DH, flush_dbg)
                    nc.compile()
                    return nc

                # ---- layer-1 flush: h=relu(dinv*agg+b1); T2'=(dinv.*h)@W2
                with (
                    tc.tile_pool(name="fl_sb", bufs=3) as fsb,
                    tc.tile_pool(name="fl_ps", bufs=2, space="PSUM") as fps,
                ):
                    def flush1(w, agg):
                        v = fsb.tile([128, DH], f32, tag="v", name="v")
                        nc.vector.tensor_scalar_mul(
                            v[:], agg, dv_s[:, w:w + 1])
                        v2 = fsb.tile([128, DH], f32, tag="v2", name="v2")
                        nc.vector.tensor_add(out=v2[:], in0=v[:], in1=b1_s[:])
                        h2 = fsb.tile([128, DH], f32, tag="h2", name="h2")
                        # dinv*relu(x) == relu(dinv*x) since dinv>0
                        nc.scalar.activation(h2[:], v2[:], func=AF.Relu,
                                             scale=dv_s[:, w:w + 1])
                        htp = fps.tile([DH, 128], f32, tag="htp", name="htp")
                        nc.tensor.transpose(htp[:], h2[:], id_s[:])
                        hts = fsb.tile([DH, 128], f32, tag="hts", name="hts")
                        nc.vector.tensor_copy(hts[:], htp[:])
                        t2p = fps.tile([128, DOUT], f32, tag="t2p", name="t2p")
                        nc.tensor.matmul(t2p[:], lhsT=hts[:], rhs=w2_s[:],
                                         start=True, stop=True)
                        t2sb = fsb.tile([128, DH], f32, tag="t2sb",
                                        name="t2sb")
                        nc.vector.memset(t2sb[:, DOUT:], 0.0)
                        nc.vector.tensor_copy(t2sb[:, :DOUT], t2p[:])
                        nc.sync.dma_start(
                            t2_shard[w * 128:(w + 1) * 128, :], t2sb[:])

                    aggregate(t1_full[:], DH, flush1)

                    # ---- phase 4: AllGather layer-2 table
                    nc.gpsimd.collective_compute(
                        "AllGather", mybir.AluOpType.bypass, replica_groups=rg,
                        ins=[t2_shard[:]], outs=[t2_full[:]],
                    )

                    if STAGE == 4:
                        with tc.tile_pool(name="dbg", bufs=2) as dbg:
                            for w in range(WPC):
                                d = dbg.tile([128, DOUT], f32, name="d")
                                nc.sync.dma_start(
                                    d[:],
                                    t2_full[w * 128:(w + 1) * 128, :DOUT])
                                d2 = dbg.tile([128, DOUT], f32, name="d2")
                                nc.vector.tensor_copy(d2[:], d[:])
                                nc.sync.dma_start(
                                    out_d.ap()[w * 128:(w + 1) * 128, :],
                                    d2[:])
                        nc.compile()
                        return nc

                    # ---- layer-2 flush: h2 + attention gate -> out
                    def flush2(w, agg):
                        v = fsb.tile([128, DOUT], f32, tag="f2v", name="f2v")
                        nc.vector.tensor_scalar_mul(
                            v[:], agg, dv_s[:, w:w + 1])
                        v2 = fsb.tile([128, DOUT], f32, tag="f2v2",
                                      name="f2v2")
                        nc.vector.tensor_add(out=v2[:], in0=v[:], in1=b2_s[:])
                        hh = fsb.tile([128, DOUT], f32, tag="f2h", name="f2h")
                        nc.scalar.activation(hh[:], v2[:], func=AF.Relu)
                        a = fsb.tile([128, DOUT], f32, tag="f2a", name="f2a")
                        nc.vector.tensor_mul(out=a[:], in0=hh[:], in1=aw_s[:])
                        ar = fsb.tile([128, 1], f32, tag="f2ar", name="f2ar")
                        nc.vector.tensor_reduce(
                            ar[:], a[:], axis=mybir.AxisListType.X,
                            op=mybir.AluOpType.add)
                        at = fsb.tile([128, 1], f32, tag="f2at", name="f2at")
                        nc.scalar.activation(at[:], ar[:], func=AF.Sigmoid,
                                             bias=ab_s[:, :1])
                        o = fsb.tile([128, DOUT], f32, tag="f2o", name="f2o")
                        nc.vector.tensor_scalar_mul(o[:], hh[:], at[:])
                        nc.sync.dma_start(
                            out_d.ap()[w * 128:(w + 1) * 128, :], o[:])

                    aggregate(t2_full[:], DOUT, flush2)

    nc.compile()
    return nc


# ---------------------------------------------------------------------------
# entry point
# ---------------------------------------------------------------------------


def _make_in_maps(cfg, host, W1, b1, W2, b2, attn_w, attn_b):
    NC = cfg.NC
    ident = np.eye(128, dtype=np.float32)
    giota = np.tile(np.arange(128, dtype=np.float32), (128, 1))
    in_maps = []
    for c in range(NC):
        in_maps.append({
            "x_sh": host["x_sh"][c],
            "w1": np.asarray(W1, np.float32),
            "w2": np.asarray(W2, np.float32),
            "b1rep": np.tile(np.asarray(b1, np.float32), (128, 1)),
            "b2rep": np.tile(np.asarray(b2, np.float32), (128, 1)),
            "awrep": np.tile(np.asarray(attn_w, np.float32).reshape(1, -1),
                             (128, 1)),
            "abcol": np.full((128, 1),
                             np.asarray(attn_b, np.float32).reshape(-1)[0],
                             np.float32),
            "dinv_slot": host["dinv_slot"][c],
            "ident": ident,
            "giota": giota,
            "idx_all": host["idx_all"][c],
            "dval_all": host["dval_all"][c],
        })
    return in_maps


def run(x, edge_index, W1, b1, W2, b2, attn_w, attn_b, cfg=None,
        backend="hw", trace=False):
    cfg = cfg or FULL
    plan, host = prep(x, edge_index, cfg)
    nc = build(cfg, plan)
    in_maps = _make_in_maps(cfg, host, W1, b1, W2, b2, attn_w, attn_b)

    if backend == "sim":
        from concourse.bass_interp import MultiCoreSim
        sim = MultiCoreSim(nc, num_cores=cfg.NC, trace=False)
        for c, core in enumerate(sim.cores.values()):
            for name, arr in in_maps[c].items():
                core.tensor(name)[:] = arr
        sim.simulate()
        outs = [core.tensor("out_sh").copy() for core in sim.cores.values()]
        exec_ns = None
    else:
        from concourse import bass_utils
        from concourse.bass_interp import get_hw_module
        old = nc.m
        nc.m = get_hw_module(nc.m)
        try:
            res = bass_utils.run_bass_kernel_spmd(
                nc, in_maps, core_ids=list(range(cfg.NC)), trace=trace)
        finally:
            nc.m = old
        outs = [res.results[c]["out_sh"] for c in range(cfg.NC)]
        exec_ns = res.exec_time_ns

    full = np.concatenate(outs, axis=0)  # [TOT, DOUT] in slot order
    out = full[host["pos"]]              # unpermute -> [N, DOUT]
    return np.ascontiguousarray(out), exec_ns


def kernel(x, edge_index, W1, b1, W2, b2, attn_w, attn_b):
    out, _ = run(x, edge_index, W1, b1, W2, b2, attn_w, attn_b,
                 cfg=FULL, backend="hw", trace=False)
    return out


# revision 23
# speedup vs baseline: 1.4573x; 1.2251x over previous
"""Trainium2 Bass kernel for a 2-layer GCN with data-aware attention gate.

Math (per reference):
    src,dst = edges + self-loops; deg = bincount(dst); dinv = rsqrt(deg)
    norm = dinv[src]*dinv[dst]
    h1 = relu(segsum(norm * (x@W1)[src], dst) + b1)
    h2 = relu(segsum(norm * (h1@W2)[src], dst) + b2)
    out = h2 * sigmoid(h2@attn_w + attn_b)

Device strategy (8 NeuronCores, node/dst-sharded):
    norm factorizes: agg[d] = dinv[d] * sum_{e->d} (dinv[s] * T[s]).
    Per layer: each core computes T' = (dinv .* H) @ W for its node shard
    (node-major via one PE transpose), AllGather of T' shards, then per-edge
    dma_gather of T' rows from HBM and PE one-hot selection-matrix matmuls
    accumulate 128-slot window segment sums in PSUM (one bank per window,
    windows strictly sequential so accumulation groups never interleave).
    Host-side prep deals nodes into windows (LPT on degree) so window loads
    are equal, and pads per-(window,parity) edge counts to the same
    128-multiple across cores so the single SPMD instruction stream is valid
    for all 8 cores.  int16 gather indices only reach 32767, so rows are
    gathered from a [TOT/2, 128]-strided view of the [TOT, 64] table
    (elem_step=128) with edges split by source-position parity.
"""

import sys

import numpy as np

_CONC = "/opt/trn_rl_repo"
if _CONC not in sys.path:
    sys.path.insert(0, _CONC)

# ---------------------------------------------------------------------------
# configuration
# ---------------------------------------------------------------------------


class Cfg:
    def __init__(self, N=50000, DIN=128, DH=64, DOUT=32, NC=8, WPC=49, WPG=7):
        self.N, self.DIN, self.DH, self.DOUT = N, DIN, DH, DOUT
        self.NC, self.WPC, self.WPG = NC, WPC, WPG
        assert WPC % WPG == 0
        self.G = WPC // WPG            # gather groups per core
        self.NPC = WPC * 128           # slots per core
        self.TOT = NC * self.NPC       # total slots
        assert self.TOT // 2 <= 32768
        assert DH * 4 == 256           # gather elem must be 256B
        assert self.N <= self.TOT - 2


FULL = Cfg()

# ---------------------------------------------------------------------------
# host-side graph prep
# ---------------------------------------------------------------------------


def _assign_slots(deg, cfg):
    """LPT-deal nodes into NC*WPC bins of <=128 slots, balancing edge load.
    Returns pos[node] -> global slot position."""
    import heapq

    nbins = cfg.NC * cfg.WPC
    cap = np.full(nbins, 128, np.int64)
    order = np.argsort(-deg, kind="stable")
    heap = [(0, b) for b in range(nbins)]
    heapq.heapify(heap)
    count = np.zeros(nbins, np.int64)
    pos = np.empty(cfg.N, np.int64)
    for n in order:
        load, b = heapq.heappop(heap)
        pos[n] = b * 128 + count[b]
        count[b] += 1
        if count[b] < cap[b]:
            heapq.heappush(heap, (load + int(deg[n]), b))
    return pos


def prep(x, edge_index, cfg):
    """Build per-core input arrays and the static (SPMD-uniform) chunk plan."""
    N, NC, WPC, WPG, G = cfg.N, cfg.NC, cfg.WPC, cfg.WPG, cfg.G
    NPC, DIN = cfg.NPC, cfg.DIN

    loops = np.arange(N, dtype=np.int64)
    src = np.concatenate([edge_index[0].astype(np.int64), loops])
    dst = np.concatenate([edge_index[1].astype(np.int64), loops])
    deg = np.bincount(dst, minlength=N).astype(np.float32)
    dinv = (1.0 / np.sqrt(np.maximum(deg, 1e-12))).astype(np.float32)

    pos = _assign_slots(deg, cfg)

    # per-core x shard + per-slot dinv
    x_sh = np.zeros((NC, NPC, DIN), np.float32)
    dinv_slot = np.ones((NC, 128, WPC), np.float32)
    node_of = np.full(cfg.TOT, -1, np.int64)
    node_of[pos] = np.arange(N)
    for c in range(NC):
        seg = node_of[c * NPC:(c + 1) * NPC]
        m = seg >= 0
        x_sh[c][m] = np.asarray(x)[seg[m]]
        dv = np.ones(NPC, np.float32)
        dv[m] = dinv[seg[m]]
        dinv_slot[c] = dv.reshape(WPC, 128).T

    # edge records
    s_pos = pos[src]
    d_pos = pos[dst]
    c_e = d_pos // NPC
    w_e = (d_pos % NPC) // 128          # window within core
    dval_e = (d_pos % 128).astype(np.float32)  # slot within window
    half_e = (s_pos & 1).astype(np.int64)
    gidx_e = (s_pos >> 1).astype(np.int64)

    # bucket edges by (core, window, half)
    from collections import defaultdict
    buckets = defaultdict(list)
    key_all = (c_e * WPC + w_e) * 2 + half_e
    order_e = np.argsort(key_all, kind="stable")
    ks = key_all[order_e]
    bounds = np.searchsorted(ks, np.arange(NC * WPC * 2 + 1))
    for key in range(NC * WPC * 2):
        lo, hi = bounds[key], bounds[key + 1]
        if hi > lo:
            buckets[key] = order_e[lo:hi]

    # per-(window,half) 128-aligned target, equalized across cores
    tgt = np.zeros((WPC, 2), np.int64)
    for w in range(WPC):
        for h in range(2):
            mx = max(len(buckets.get((c * WPC + w) * 2 + h, ()))
                     for c in range(NC))
            tgt[w, h] = int(np.ceil(max(mx, 1) / 128) * 128)

    # per-(group,half) gather segment = concat of member windows' segments
    seglen = np.zeros((G, 2), np.int64)
    for g in range(G):
        for h in range(2):
            seglen[g, h] = tgt[g * WPG:(g + 1) * WPG, h].sum()

    idx_cols = int(sum(seglen[g, h] // 16 for g in range(G) for h in range(2)))
    chunk_tot = int(sum(seglen[g, h] // 128 for g in range(G) for h in range(2)))
    idx_all = np.zeros((NC, 128, idx_cols), np.int16)
    dval_all = np.full((NC, 128, chunk_tot), -1.0, np.float32)

    ioff, coff = {}, {}
    io = co = 0
    for g in range(G):
        for h in range(2):
            ioff[(g, h)] = io
            coff[(g, h)] = co
            io += int(seglen[g, h]) // 16
            co += int(seglen[g, h]) // 128
    # chunk column (within dval_all / gather tile) of window w's half-h run
    wcol = np.zeros((WPC, 2), np.int64)
    for g in range(G):
        for h in range(2):
            c0 = coff[(g, h)]
            for wl in range(WPG):
                w = g * WPG + wl
                wcol[w, h] = c0
                c0 += tgt[w, h] // 128

    for c in range(NC):
        for g in range(G):
            for h in range(2):
                n = int(seglen[g, h])
                gi = np.zeros(n, np.int64)
                dv = np.full(n, -1.0, np.float32)
                p = 0
                for wl in range(WPG):
                    w = g * WPG + wl
                    es = buckets.get((c * WPC + w) * 2 + h, ())
                    ne = len(es)
                    gi[p:p + ne] = gidx_e[es]
                    dv[p:p + ne] = dval_e[es]
                    p += int(tgt[w, h])
                wrapped = gi.reshape(n // 16, 16).T.astype(np.int16)
                idx_all[c, :, ioff[(g, h)]:ioff[(g, h)] + n // 16] = np.tile(
                    wrapped, (8, 1))
                dval_all[c, :, coff[(g, h)]:coff[(g, h)] + n // 128] = (
                    dv.reshape(n // 128, 128).T)

    plan = dict(tgt=tgt, seglen=seglen, ioff=ioff, coff=coff, wcol=wcol,
                idx_cols=idx_cols, chunk_tot=chunk_tot)
    host = dict(x_sh=x_sh, dinv_slot=dinv_slot, idx_all=idx_all,
                dval_all=dval_all, pos=pos)
    return plan, host


# ---------------------------------------------------------------------------
# device kernel
# ---------------------------------------------------------------------------


def build(cfg, plan):
    import os
    import concourse.bass as bass
    import concourse.mybir as mybir
    import concourse.tile as tile
    from concourse import bacc

    STAGE = int(os.environ.get("KERNEL_STAGE", "5"))
    NOGATHER = os.environ.get("KERNEL_NOGATHER", "0") == "1"
    NOCOLL = os.environ.get("KERNEL_NOCOLL", "0") == "1"

    NC, WPC, WPG, G = cfg.NC, cfg.WPC, cfg.WPG, cfg.G
    NPC, TOT, DIN, DH, DOUT = cfg.NPC, cfg.TOT, cfg.DIN, cfg.DH, cfg.DOUT
    f32 = mybir.dt.float32
    bf16 = mybir.dt.bfloat16
    tgt, seglen = plan["tgt"], plan["seglen"]
    ioff, coff, wcol = plan["ioff"], plan["coff"], plan["wcol"]
    AF = mybir.ActivationFunctionType

    nc = bacc.Bacc(
        "TRN2", target_bir_lowering=False, debug=False,
        num_devices=NC, num_swdge_queues=4,
    )

    # I/O
    x_d = nc.dram_tensor("x_sh", [NPC, DIN], f32, kind="ExternalInput")
    w1_d = nc.dram_tensor("w1", [DIN, DH], f32, kind="ExternalInput")
    w2_d = nc.dram_tensor("w2", [DH, DOUT], f32, kind="ExternalInput")
    b1_d = nc.dram_tensor("b1rep", [128, DH], f32, kind="ExternalInput")
    b2_d = nc.dram_tensor("b2rep", [128, DOUT], f32, kind="ExternalInput")
    aw_d = nc.dram_tensor("awrep", [128, DOUT], f32, kind="ExternalInput")
    ab_d = nc.dram_tensor("abcol", [128, 1], f32, kind="ExternalInput")
    dv_d = nc.dram_tensor("dinv_slot", [128, WPC], f32, kind="ExternalInput")
    id_d = nc.dram_tensor("ident", [128, 128], f32, kind="ExternalInput")
    gi_d = nc.dram_tensor("giota", [128, 128], bf16, kind="ExternalInput")
    ix_d = nc.dram_tensor("idx_all", [128, plan["idx_cols"]], mybir.dt.int16,
                          kind="ExternalInput")
    dvl_d = nc.dram_tensor("dval_all", [128, plan["chunk_tot"]], f32,
                           kind="ExternalInput")
    out_d = nc.dram_tensor("out_sh", [NPC, DOUT], f32, kind="ExternalOutput")

    rg = [list(range(NC))]

    with tile.TileContext(nc) as tc:
        with tc.tile_pool(name="const", bufs=1) as cpool:
            def load(dram, shape, dt=f32):
                t = cpool.tile(shape, dt, tag=dram.name, name=dram.name + "_s")
                nc.sync.dma_start(t[:], dram.ap())
                return t

            w1_s = load(w1_d, [DIN, DH])
            w2_s = load(w2_d, [DH, DOUT])
            b1_s = load(b1_d, [128, DH])
            b2_s = load(b2_d, [128, DOUT])
            aw_s = load(aw_d, [128, DOUT])
            ab_s = load(ab_d, [128, 1])
            dv_s = load(dv_d, [128, WPC])
            id_s = load(id_d, [128, 128])
            gi_s = load(gi_d, [128, 128], bf16)
            ix_s = load(ix_d, [128, plan["idx_cols"]], mybir.dt.int16)
            dvl_s = load(dvl_d, [128, plan["chunk_tot"]])

            with tc.tile_pool(name="dram", bufs=1, space="DRAM") as dpool:
                t1_shard = dpool.tile([NPC, 2 * DH], bf16, tag="t1s",
                                      name="t1s")
                t1_full = dpool.tile([TOT, 2 * DH], bf16, tag="t1f",
                                     name="t1f", addr_space="Shared")
                t2_shard = dpool.tile([NPC, 2 * DH], bf16, tag="t2s",
                                      name="t2s")
                t2_full = dpool.tile([TOT, 2 * DH], bf16, tag="t2f",
                                     name="t2f", addr_space="Shared")

                # ---- phase 1: T1' = (dinv .* x) @ W1, node-major
                with (
                    tc.tile_pool(name="tf_in", bufs=3) as pin,
                    tc.tile_pool(name="tf_ps", bufs=2, space="PSUM") as pps,
                    tc.tile_pool(name="tf_sb", bufs=3) as psb,
                ):
                    for w in range(WPC):
                        xt = pin.tile([128, DIN], f32, tag="xt", name="xt")
                        nc.sync.dma_start(
                            xt[:], x_d.ap()[w * 128:(w + 1) * 128, :])
                        xs = psb.tile([128, DIN], f32, tag="xs", name="xs")
                        nc.vector.tensor_scalar_mul(
                            xs[:], xt[:], dv_s[:, w:w + 1])
                        xtp = pps.tile([128, DIN], f32, tag="xtp", name="xtp")
                        nc.tensor.transpose(xtp[:], xs[:], id_s[:])
                        xts = psb.tile([128, DIN], f32, tag="xts", name="xts")
                        nc.vector.tensor_copy(xts[:], xtp[:])
                        hp = pps.tile([128, DH], f32, tag="hp", name="hp")
                        nc.tensor.matmul(hp[:], lhsT=xts[:], rhs=w1_s[:],
                                         start=True, stop=True)
                        hf = psb.tile([128, DH], f32, tag="hf", name="hf")
                        nc.vector.tensor_copy(hf[:], hp[:])
                        hs = psb.tile([128, 2 * DH], bf16, tag="hs", name="hs")
                        nc.vector.tensor_copy(hs[:, :DH], hf[:])
                        hib = psb.tile([128, DH], f32, tag="hib", name="hib")
                        nc.vector.tensor_copy(hib[:], hs[:, :DH])
                        nc.vector.tensor_tensor(
                            out=hs[:, DH:], in0=hf[:], in1=hib[:],
                            op=mybir.AluOpType.subtract)
                        nc.sync.dma_start(
                            t1_shard[w * 128:(w + 1) * 128, :], hs[:])

                if STAGE == 1:
                    # debug: dump T1' shard columns into out and stop
                    with tc.tile_pool(name="dbg", bufs=2) as dbg:
                        for w in range(WPC):
                            d = dbg.tile([128, DOUT], f32, name="d")
                            nc.sync.dma_start(
                                d[:],
                                t1_shard[w * 128:(w + 1) * 128, :DOUT])
                            d2 = dbg.tile([128, DOUT], f32, name="d2")
                            nc.vector.tensor_copy(d2[:], d[:])
                            nc.sync.dma_start(
                                out_d.ap()[w * 128:(w + 1) * 128, :], d2[:])
                    nc.compile()
                    return nc

                # ---- phase 2: AllGather layer-1 table
                if NOCOLL:
                    nc.sync.dma_start(t1_full[0:NPC, :], t1_shard[:])
                else:
                    nc.gpsimd.collective_compute(
                        "AllGather", mybir.AluOpType.bypass, replica_groups=rg,
                        ins=[t1_shard[:]], outs=[t1_full[:]],
                    )

                if STAGE == 2:
                    with tc.tile_pool(name="dbg", bufs=2) as dbg:
                        for w in range(WPC):
                            d = dbg.tile([128, DOUT], f32, name="d")
                            nc.sync.dma_start(
                                d[:], t1_full[w * 128:(w + 1) * 128, :DOUT])
                            d2 = dbg.tile([128, DOUT], f32, name="d2")
                            nc.vector.tensor_copy(d2[:], d[:])
                            nc.sync.dma_start(
                                out_d.ap()[w * 128:(w + 1) * 128, :], d2[:])
                    nc.compile()
                    return nc

                # ---- aggregation: gather rows + one-hot matmul segment sums
                def aggregate(full, ncols, flush_fn, qctr=[0]):
                    EW = 2 * DH  # bf16 row width (hi | lo)
                    fv = full.rearrange("(a b) d -> a (b d)", b=2)
                    with (
                        tc.tile_pool(name="gpool", bufs=2) as gp,
                        tc.tile_pool(name="spool", bufs=6) as sp,
                        tc.tile_pool(name="apsum", bufs=4, space="PSUM") as aps,
                    ):
                        for g in range(G):
                            gts = {}
                            for h in range(2):
                                n = int(seglen[g, h])
                                nch = n // 128
                                gt = gp.tile([128, nch * EW], bf16,
                                             tag=f"g{h}", name=f"gt{h}")
                                io = ioff[(g, h)]
                                if NOGATHER:
                                    nc.sync.dma_start(
                                        gt[:],
                                        full[0:128 * nch, :].rearrange(
                                            "(p c) d -> p (c d)", p=128))
                                else:
                                    nc.gpsimd.dma_gather(
                                        out_ap=gt[:].rearrange(
                                            "p (c d) -> p c d", d=EW),
                                        in_ap=fv[:, h * EW:(h + 1) * EW],
                                        idxs_ap=ix_s[:, io:io + n // 16],
                                        num_idxs=n, num_idxs_reg=n,
                                        elem_size=EW, elem_step=2 * EW,
                                        queue_num=qctr[0] % 4,
                                        single_packet=False,
                                    )
                                    qctr[0] += 1
                                gts[h] = gt
                            for wl in range(WPG):
                                w = g * WPG + wl
                                ps = aps.tile([128, DH], f32, tag="agg",
                                              name="agg")
                                chunks = (
                                    [(0, k) for k in range(int(tgt[w, 0]) // 128)]
                                    + [(1, k) for k in range(int(tgt[w, 1]) // 128)]
                                )
                                for j, (h, k) in enumerate(chunks):
                                    tcol = int(wcol[w, h] - coff[(g, h)]) + k
                                    dcol = int(wcol[w, h]) + k
                                    S = sp.tile([128, 128], bf16, tag="S",
                                                name="S")
                                    nc.vector.tensor_scalar(
                                        out=S[:],
                                        in0=gi_s[:],
                                        scalar1=dvl_s[:, dcol:dcol + 1],
                                        scalar2=None,
                                        op0=mybir.AluOpType.is_equal,
                                    )
                                    base = tcol * EW
                                    nc.tensor.matmul(
                                        ps[:, :ncols],
                                        lhsT=S[:],
                                        rhs=gts[h][:, base:base + ncols],
                                        start=(j == 0), stop=False,
                                    )
                                    nc.tensor.matmul(
                                        ps[:, :ncols],
                                        lhsT=S[:],
                                        rhs=gts[h][:, base + DH:base + DH + ncols],
                                        start=False,
                                        stop=(j == len(chunks) - 1),
                                    )
                                flush_fn(w, ps[:, :ncols])

                if STAGE == 3:
                    with tc.tile_pool(name="dbg", bufs=2) as dbg:
                        def flush_dbg(w, agg):
                            d2 = dbg.tile([128, DOUT], f32, name="d2")
                            nc.vector.tensor_copy(d2[:], agg[:, :DOUT])
                            nc.sync.dma_start(
                                out_d.ap()[w * 128:(w + 1) * 128, :], d2[:])
                        aggregate(t1_full[:], DH, flush_dbg)
                    nc.compile()
                    return nc

                # ---- layer-1 flush: h=relu(dinv*agg+b1); T2'=(dinv.*h)@W2
                with (
                    tc.tile_pool(name="fl_sb", bufs=3) as fsb,
                    tc.tile_pool(name="fl_ps", bufs=2, space="PSUM") as fps,
                ):
                    def flush1(w, agg):
                        v = fsb.tile([128, DH], f32, tag="v", name="v")
                        nc.vector.tensor_scalar_mul(
                            v[:], agg, dv_s[:, w:w + 1])
                        v2 = fsb.tile([128, DH], f32, tag="v2", name="v2")
                        nc.vector.tensor_add(out=v2[:], in0=v[:], in1=b1_s[:])
                        h2 = fsb.tile([128, DH], f32, tag="h2", name="h2")
                        # dinv*relu(x) == relu(dinv*x) since dinv>0
                        nc.scalar.activation(h2[:], v2[:], func=AF.Relu,
                                             scale=dv_s[:, w:w + 1])
                        htp = fps.tile([DH, 128], f32, tag="htp", name="htp")
                        nc.tensor.transpose(htp[:], h2[:], id_s[:])
                        hts = fsb.tile([DH, 128], f32, tag="hts", name="hts")
                        nc.vector.tensor_copy(hts[:], htp[:])
                        t2p = fps.tile([128, DOUT], f32, tag="t2p", name="t2p")
                        nc.tensor.matmul(t2p[:], lhsT=hts[:], rhs=w2_s[:],
                                         start=True, stop=True)
                        t2f = fsb.tile([128, DOUT], f32, tag="t2f32",
                                       name="t2f32")
                        nc.vector.tensor_copy(t2f[:], t2p[:])
                        # layout: hi at [0,DOUT), lo at [DH,DH+DOUT)
                        t2sb = fsb.tile([128, 2 * DH], bf16, tag="t2sb",
                                        name="t2sb")
                        nc.vector.memset(t2sb[:, DOUT:DH], 0.0)
                        nc.vector.memset(t2sb[:, DH + DOUT:], 0.0)
                        nc.vector.tensor_copy(t2sb[:, :DOUT], t2f[:])
                        t2ib = fsb.tile([128, DOUT], f32, tag="t2ib",
                                        name="t2ib")
                        nc.vector.tensor_copy(t2ib[:], t2sb[:, :DOUT])
                        nc.vector.tensor_tensor(
                            out=t2sb[:, DH:DH + DOUT], in0=t2f[:],
                            in1=t2ib[:], op=mybir.AluOpType.subtract)
                        nc.sync.dma_start(
                            t2_shard[w * 128:(w + 1) * 128, :], t2sb[:])

                    aggregate(t1_full[:], DH, flush1)

                    # ---- phase 4: AllGather layer-2 table
                    nc.gpsimd.collective_compute(
                        "AllGather", mybir.AluOpType.bypass, replica_groups=rg,
                        ins=[t2_shard[:]], outs=[t2_full[:]],
                    )

                    if STAGE == 4:
                        with tc.tile_pool(name="dbg", bufs=2) as dbg:
                            for w in range(WPC):
                                d = dbg.tile([128, DOUT], f32, name="d")
                                nc.sync.dma_start(
                                    d[:],
                                    t2_full[w * 128:(w + 1) * 128, :DOUT])
                                d2 = dbg.tile([128, DOUT], f32, name="d2")
                                nc.vector.tensor_copy(d2[:], d[:])
                                nc.sync.dma_start(
                                    out_d.ap()[w * 128:(w + 1) * 128, :],
                                    d2[:])
                        nc.compile()
                        return nc

                    # ---- layer-2 flush: h2 + attention gate -> out
                    def flush2(w, agg):
                        v = fsb.tile([128, DOUT], f32, tag="f2v", name="f2v")
                        nc.vector.tensor_scalar_mul(
                            v[:], agg, dv_s[:, w:w + 1])
                        v2 = fsb.tile([128, DOUT], f32, tag="f2v2",
                                      name="f2v2")
                        nc.vector.tensor_add(out=v2[:], in0=v[:], in1=b2_s[:])
                        hh = fsb.tile([128, DOUT], f32, tag="f2h", name="f2h")
                        nc.scalar.activation(hh[:], v2[:], func=AF.Relu)
                        a = fsb.tile([128, DOUT], f32, tag="f2a", name="f2a")
                        nc.vector.tensor_mul(out=a[:], in0=hh[:], in1=aw_s[:])
                        ar = fsb.tile([128, 1], f32, tag="f2ar", name="f2ar")
                        nc.vector.tensor_reduce(
                            ar[:], a[:], axis=mybir.AxisListType.X,
                            op=mybir.AluOpType.add)
                        at = fsb.tile([128, 1], f32, tag="f2at", name="f2at")
                        nc.scalar.activation(at[:], ar[:], func=AF.Sigmoid,
                                             bias=ab_s[:, :1])
                        o = fsb.tile([128, DOUT], f32, tag="f2o", name="f2o")
                        nc.vector.tensor_scalar_mul(o[:], hh[:], at[:])
                        nc.sync.dma_start(
                            out_d.ap()[w * 128:(w + 1) * 128, :], o[:])

                    aggregate(t2_full[:], DOUT, flush2)

    nc.compile()
    return nc


# ---------------------------------------------------------------------------
# entry point
# ---------------------------------------------------------------------------


def _make_in_maps(cfg, host, W1, b1, W2, b2, attn_w, attn_b):
    import ml_dtypes
    NC = cfg.NC
    bf16 = ml_dtypes.bfloat16
    ident = np.eye(128, dtype=np.float32)
    giota = np.tile(np.arange(128, dtype=np.float32), (128, 1)).astype(bf16)
    in_maps = []
    for c in range(NC):
        in_maps.append({
            "x_sh": host["x_sh"][c],
            "w1": np.asarray(W1, np.float32),
            "w2": np.asarray(W2, np.float32),
            "b1rep": np.tile(np.asarray(b1, np.float32), (128, 1)),
            "b2rep": np.tile(np.asarray(b2, np.float32), (128, 1)),
            "awrep": np.tile(np.asarray(attn_w, np.float32).reshape(1, -1),
                             (128, 1)),
            "abcol": np.full((128, 1),
                             np.asarray(attn_b, np.float32).reshape(-1)[0],
                             np.float32),
            "dinv_slot": host["dinv_slot"][c],
            "ident": ident,
            "giota": giota,
            "idx_all": host["idx_all"][c],
            "dval_all": host["dval_all"][c],
        })
    return in_maps


def run(x, edge_index, W1, b1, W2, b2, attn_w, attn_b, cfg=None,
        backend="hw", trace=False):
    cfg = cfg or FULL
    plan, host = prep(x, edge_index, cfg)
    nc = build(cfg, plan)
    in_maps = _make_in_maps(cfg, host, W1, b1, W2, b2, attn_w, attn_b)

    if backend == "sim":
        from concourse.bass_interp import MultiCoreSim
        sim = MultiCoreSim(nc, num_cores=cfg.NC, trace=False)
        for c, core in enumerate(sim.cores.values()):
            for name, arr in in_maps[c].items():
                core.tensor(name)[:] = arr
        sim.simulate()
        outs = [core.tensor("out_sh").copy() for core in sim.cores.values()]
        exec_ns = None
    else:
        from concourse import bass_utils
        from concourse.bass_interp import get_hw_module
        old = nc.m
        nc.m = get_hw_module(nc.m)
        try:
            res = bass_utils.run_bass_kernel_spmd(
                nc, in_maps, core_ids=list(range(cfg.NC)), trace=trace)
        finally:
            nc.m = old
        outs = [res.results[c]["out_sh"] for c in range(cfg.NC)]
        exec_ns = res.exec_time_ns

    full = np.concatenate(outs, axis=0)  # [TOT, DOUT] in slot order
    out = full[host["pos"]]              # unpermute -> [N, DOUT]
    return np.ascontiguousarray(out), exec_ns


def kernel(x, edge_index, W1, b1, W2, b2, attn_w, attn_b):
    out, _ = run(x, edge_index, W1, b1, W2, b2, attn_w, attn_b,
                 cfg=FULL, backend="hw", trace=False)
    return out


# revision 24
# speedup vs baseline: 1.7078x; 1.1719x over previous
"""Trainium2 Bass kernel for a 2-layer GCN with data-aware attention gate.

Math (per reference):
    src,dst = edges + self-loops; deg = bincount(dst); dinv = rsqrt(deg)
    norm = dinv[src]*dinv[dst]
    h1 = relu(segsum(norm * (x@W1)[src], dst) + b1)
    h2 = relu(segsum(norm * (h1@W2)[src], dst) + b2)
    out = h2 * sigmoid(h2@attn_w + attn_b)

Device strategy (8 NeuronCores, node/dst-sharded):
    norm factorizes: agg[d] = dinv[d] * sum_{e->d} (dinv[s] * T[s]).
    Per layer: each core computes T' = (dinv .* H) @ W for its node shard
    (node-major via one PE transpose), AllGather of T' shards, then per-edge
    dma_gather of T' rows from HBM and PE one-hot selection-matrix matmuls
    accumulate 128-slot window segment sums in PSUM (one bank per window,
    windows strictly sequential so accumulation groups never interleave).
    Host-side prep deals nodes into windows (LPT on degree) so window loads
    are equal, and pads per-(window,parity) edge counts to the same
    128-multiple across cores so the single SPMD instruction stream is valid
    for all 8 cores.  int16 gather indices only reach 32767, so rows are
    gathered from a [TOT/2, 128]-strided view of the [TOT, 64] table
    (elem_step=128) with edges split by source-position parity.
"""

import sys

import numpy as np

_CONC = "/opt/trn_rl_repo"
if _CONC not in sys.path:
    sys.path.insert(0, _CONC)

# ---------------------------------------------------------------------------
# configuration
# ---------------------------------------------------------------------------


class Cfg:
    def __init__(self, N=50000, DIN=128, DH=64, DOUT=32, NC=8, WPC=49, WPG=7):
        self.N, self.DIN, self.DH, self.DOUT = N, DIN, DH, DOUT
        self.NC, self.WPC, self.WPG = NC, WPC, WPG
        assert WPC % WPG == 0
        self.G = WPC // WPG            # gather groups per core
        self.NPC = WPC * 128           # slots per core
        self.TOT = NC * self.NPC       # total slots
        assert self.TOT // 2 <= 32768
        assert DH * 4 == 256           # gather elem must be 256B
        assert self.N <= self.TOT - 2


FULL = Cfg()

# ---------------------------------------------------------------------------
# host-side graph prep
# ---------------------------------------------------------------------------


def _assign_slots(deg, cfg):
    """LPT-deal nodes into NC*WPC bins of <=128 slots, balancing edge load.
    Returns pos[node] -> global slot position."""
    import heapq

    nbins = cfg.NC * cfg.WPC
    cap = np.full(nbins, 128, np.int64)
    order = np.argsort(-deg, kind="stable")
    heap = [(0, b) for b in range(nbins)]
    heapq.heapify(heap)
    count = np.zeros(nbins, np.int64)
    pos = np.empty(cfg.N, np.int64)
    for n in order:
        load, b = heapq.heappop(heap)
        pos[n] = b * 128 + count[b]
        count[b] += 1
        if count[b] < cap[b]:
            heapq.heappush(heap, (load + int(deg[n]), b))
    return pos


def prep(x, edge_index, cfg):
    """Build per-core input arrays and the static (SPMD-uniform) chunk plan."""
    N, NC, WPC, WPG, G = cfg.N, cfg.NC, cfg.WPC, cfg.WPG, cfg.G
    NPC, DIN = cfg.NPC, cfg.DIN

    loops = np.arange(N, dtype=np.int64)
    src = np.concatenate([edge_index[0].astype(np.int64), loops])
    dst = np.concatenate([edge_index[1].astype(np.int64), loops])
    deg = np.bincount(dst, minlength=N).astype(np.float32)
    dinv = (1.0 / np.sqrt(np.maximum(deg, 1e-12))).astype(np.float32)

    pos = _assign_slots(deg, cfg)

    # per-core x shard + per-slot dinv
    x_sh = np.zeros((NC, NPC, DIN), np.float32)
    dinv_slot = np.ones((NC, 128, WPC), np.float32)
    node_of = np.full(cfg.TOT, -1, np.int64)
    node_of[pos] = np.arange(N)
    for c in range(NC):
        seg = node_of[c * NPC:(c + 1) * NPC]
        m = seg >= 0
        x_sh[c][m] = np.asarray(x)[seg[m]]
        dv = np.ones(NPC, np.float32)
        dv[m] = dinv[seg[m]]
        dinv_slot[c] = dv.reshape(WPC, 128).T

    # edge records
    s_pos = pos[src]
    d_pos = pos[dst]
    c_e = d_pos // NPC
    w_e = (d_pos % NPC) // 128          # window within core
    dval_e = (d_pos % 128 + 2).astype(np.float32)  # slot-in-window + 2
    half_e = (s_pos & 1).astype(np.int64)
    gidx_e = (s_pos >> 1).astype(np.int64)

    # bucket edges by (core, window, half)
    from collections import defaultdict
    buckets = defaultdict(list)
    key_all = (c_e * WPC + w_e) * 2 + half_e
    order_e = np.argsort(key_all, kind="stable")
    ks = key_all[order_e]
    bounds = np.searchsorted(ks, np.arange(NC * WPC * 2 + 1))
    for key in range(NC * WPC * 2):
        lo, hi = bounds[key], bounds[key + 1]
        if hi > lo:
            buckets[key] = order_e[lo:hi]

    # per-(window,half) 128-aligned target, equalized across cores
    tgt = np.zeros((WPC, 2), np.int64)
    for w in range(WPC):
        for h in range(2):
            mx = max(len(buckets.get((c * WPC + w) * 2 + h, ()))
                     for c in range(NC))
            tgt[w, h] = int(np.ceil(max(mx, 1) / 128) * 128)

    # per-(group,half) gather segment = concat of member windows' segments
    seglen = np.zeros((G, 2), np.int64)
    for g in range(G):
        for h in range(2):
            seglen[g, h] = tgt[g * WPG:(g + 1) * WPG, h].sum()

    idx_cols = int(sum(seglen[g, h] // 16 for g in range(G) for h in range(2)))
    chunk_tot = int(sum(seglen[g, h] // 128 for g in range(G) for h in range(2)))
    idx_all = np.zeros((NC, 128, idx_cols), np.int16)
    dval_all = np.full((NC, 128, chunk_tot), -1.0, np.float32)

    ioff, coff = {}, {}
    io = co = 0
    for g in range(G):
        for h in range(2):
            ioff[(g, h)] = io
            coff[(g, h)] = co
            io += int(seglen[g, h]) // 16
            co += int(seglen[g, h]) // 128
    # chunk column (within dval_all / gather tile) of window w's half-h run
    wcol = np.zeros((WPC, 2), np.int64)
    for g in range(G):
        for h in range(2):
            c0 = coff[(g, h)]
            for wl in range(WPG):
                w = g * WPG + wl
                wcol[w, h] = c0
                c0 += tgt[w, h] // 128

    for c in range(NC):
        for g in range(G):
            for h in range(2):
                n = int(seglen[g, h])
                gi = np.zeros(n, np.int64)
                dv = np.full(n, -1.0, np.float32)
                p = 0
                for wl in range(WPG):
                    w = g * WPG + wl
                    es = buckets.get((c * WPC + w) * 2 + h, ())
                    ne = len(es)
                    gi[p:p + ne] = gidx_e[es]
                    dv[p:p + ne] = dval_e[es]
                    p += int(tgt[w, h])
                wrapped = gi.reshape(n // 16, 16).T.astype(np.int16)
                idx_all[c, :, ioff[(g, h)]:ioff[(g, h)] + n // 16] = np.tile(
                    wrapped, (8, 1))
                dval_all[c, :, coff[(g, h)]:coff[(g, h)] + n // 128] = (
                    dv.reshape(n // 128, 128).T)

    plan = dict(tgt=tgt, seglen=seglen, ioff=ioff, coff=coff, wcol=wcol,
                idx_cols=idx_cols, chunk_tot=chunk_tot)
    host = dict(x_sh=x_sh, dinv_slot=dinv_slot, idx_all=idx_all,
                dval_all=dval_all, pos=pos)
    return plan, host


# ---------------------------------------------------------------------------
# device kernel
# ---------------------------------------------------------------------------


def build(cfg, plan):
    import os
    import concourse.bass as bass
    import concourse.mybir as mybir
    import concourse.tile as tile
    from concourse import bacc

    STAGE = int(os.environ.get("KERNEL_STAGE", "5"))
    NOGATHER = os.environ.get("KERNEL_NOGATHER", "0") == "1"
    NOCOLL = os.environ.get("KERNEL_NOCOLL", "0") == "1"

    NC, WPC, WPG, G = cfg.NC, cfg.WPC, cfg.WPG, cfg.G
    NPC, TOT, DIN, DH, DOUT = cfg.NPC, cfg.TOT, cfg.DIN, cfg.DH, cfg.DOUT
    f32 = mybir.dt.float32
    bf16 = mybir.dt.bfloat16
    tgt, seglen = plan["tgt"], plan["seglen"]
    ioff, coff, wcol = plan["ioff"], plan["coff"], plan["wcol"]
    AF = mybir.ActivationFunctionType
    AF_Relu = AF.Relu

    nc = bacc.Bacc(
        "TRN2", target_bir_lowering=False, debug=False,
        num_devices=NC, num_swdge_queues=4,
    )

    # I/O
    x_d = nc.dram_tensor("x_sh", [NPC, DIN], f32, kind="ExternalInput")
    w1_d = nc.dram_tensor("w1", [DIN, DH], f32, kind="ExternalInput")
    w2_d = nc.dram_tensor("w2", [DH, DOUT], f32, kind="ExternalInput")
    b1_d = nc.dram_tensor("b1rep", [128, DH], f32, kind="ExternalInput")
    b2_d = nc.dram_tensor("b2rep", [128, DOUT], f32, kind="ExternalInput")
    aw_d = nc.dram_tensor("awrep", [128, DOUT], f32, kind="ExternalInput")
    ab_d = nc.dram_tensor("abcol", [128, 1], f32, kind="ExternalInput")
    dv_d = nc.dram_tensor("dinv_slot", [128, WPC], f32, kind="ExternalInput")
    id_d = nc.dram_tensor("ident", [128, 128], f32, kind="ExternalInput")
    gi_d = nc.dram_tensor("giota", [128, 128], bf16, kind="ExternalInput")
    ix_d = nc.dram_tensor("idx_all", [128, plan["idx_cols"]], mybir.dt.int16,
                          kind="ExternalInput")
    dvl_d = nc.dram_tensor("dval_all", [128, plan["chunk_tot"]], f32,
                           kind="ExternalInput")
    out_d = nc.dram_tensor("out_sh", [NPC, DOUT], f32, kind="ExternalOutput")

    rg = [list(range(NC))]

    with tile.TileContext(nc) as tc:
        with tc.tile_pool(name="const", bufs=1) as cpool:
            def load(dram, shape, dt=f32):
                t = cpool.tile(shape, dt, tag=dram.name, name=dram.name + "_s")
                nc.sync.dma_start(t[:], dram.ap())
                return t

            w1_s = load(w1_d, [DIN, DH])
            w2_s = load(w2_d, [DH, DOUT])
            b1_s = load(b1_d, [128, DH])
            b2_s = load(b2_d, [128, DOUT])
            aw_s = load(aw_d, [128, DOUT])
            ab_s = load(ab_d, [128, 1])
            dv_s = load(dv_d, [128, WPC])
            id_s = load(id_d, [128, 128])
            gi_s = load(gi_d, [128, 128], bf16)
            ix_s = load(ix_d, [128, plan["idx_cols"]], mybir.dt.int16)
            dvl_s = load(dvl_d, [128, plan["chunk_tot"]])

            with tc.tile_pool(name="dram", bufs=1, space="DRAM") as dpool:
                t1_shard = dpool.tile([NPC, 2 * DH], bf16, tag="t1s",
                                      name="t1s")
                t1_full = dpool.tile([TOT, 2 * DH], bf16, tag="t1f",
                                     name="t1f", addr_space="Shared")
                t2_shard = dpool.tile([NPC, 2 * DH], bf16, tag="t2s",
                                      name="t2s")
                t2_full = dpool.tile([TOT, 2 * DH], bf16, tag="t2f",
                                     name="t2f", addr_space="Shared")

                # ---- phase 1: T1' = (dinv .* x) @ W1, node-major
                with (
                    tc.tile_pool(name="tf_in", bufs=3) as pin,
                    tc.tile_pool(name="tf_ps", bufs=2, space="PSUM") as pps,
                    tc.tile_pool(name="tf_sb", bufs=3) as psb,
                ):
                    for w in range(WPC):
                        xt = pin.tile([128, DIN], f32, tag="xt", name="xt")
                        nc.sync.dma_start(
                            xt[:], x_d.ap()[w * 128:(w + 1) * 128, :])
                        xs = psb.tile([128, DIN], f32, tag="xs", name="xs")
                        nc.vector.tensor_scalar_mul(
                            xs[:], xt[:], dv_s[:, w:w + 1])
                        xtp = pps.tile([128, DIN], f32, tag="xtp", name="xtp")
                        nc.tensor.transpose(xtp[:], xs[:], id_s[:])
                        xts = psb.tile([128, DIN], f32, tag="xts", name="xts")
                        nc.vector.tensor_copy(xts[:], xtp[:])
                        hp = pps.tile([128, DH], f32, tag="hp", name="hp")
                        nc.tensor.matmul(hp[:], lhsT=xts[:], rhs=w1_s[:],
                                         start=True, stop=True)
                        hf = psb.tile([128, DH], f32, tag="hf", name="hf")
                        nc.vector.tensor_copy(hf[:], hp[:])
                        hs = psb.tile([128, 2 * DH], bf16, tag="hs", name="hs")
                        nc.vector.tensor_copy(hs[:, :DH], hf[:])
                        hib = psb.tile([128, DH], f32, tag="hib", name="hib")
                        nc.vector.tensor_copy(hib[:], hs[:, :DH])
                        nc.vector.tensor_tensor(
                            out=hs[:, DH:], in0=hf[:], in1=hib[:],
                            op=mybir.AluOpType.subtract)
                        nc.sync.dma_start(
                            t1_shard[w * 128:(w + 1) * 128, :], hs[:])

                if STAGE == 1:
                    # debug: dump T1' shard columns into out and stop
                    with tc.tile_pool(name="dbg", bufs=2) as dbg:
                        for w in range(WPC):
                            d = dbg.tile([128, DOUT], f32, name="d")
                            nc.sync.dma_start(
                                d[:],
                                t1_shard[w * 128:(w + 1) * 128, :DOUT])
                            d2 = dbg.tile([128, DOUT], f32, name="d2")
                            nc.vector.tensor_copy(d2[:], d[:])
                            nc.sync.dma_start(
                                out_d.ap()[w * 128:(w + 1) * 128, :], d2[:])
                    nc.compile()
                    return nc

                # ---- phase 2: AllGather layer-1 table
                if NOCOLL:
                    nc.sync.dma_start(t1_full[0:NPC, :], t1_shard[:])
                else:
                    nc.gpsimd.collective_compute(
                        "AllGather", mybir.AluOpType.bypass, replica_groups=rg,
                        ins=[t1_shard[:]], outs=[t1_full[:]],
                    )

                if STAGE == 2:
                    with tc.tile_pool(name="dbg", bufs=2) as dbg:
                        for w in range(WPC):
                            d = dbg.tile([128, DOUT], f32, name="d")
                            nc.sync.dma_start(
                                d[:], t1_full[w * 128:(w + 1) * 128, :DOUT])
                            d2 = dbg.tile([128, DOUT], f32, name="d2")
                            nc.vector.tensor_copy(d2[:], d[:])
                            nc.sync.dma_start(
                                out_d.ap()[w * 128:(w + 1) * 128, :], d2[:])
                    nc.compile()
                    return nc

                # ---- aggregation: gather rows + one-hot matmul segment sums
                def aggregate(full, ncols, flush_fn, qctr=[0]):
                    EW = 2 * DH  # bf16 row width (hi | lo)
                    fv = full.rearrange("(a b) d -> a (b d)", b=2)
                    with (
                        tc.tile_pool(name="gpool", bufs=3) as gp,
                        tc.tile_pool(name="spool", bufs=6) as sp,
                        tc.tile_pool(name="bpool", bufs=6) as bp,
                        tc.tile_pool(name="apsum", bufs=4, space="PSUM") as aps,
                    ):
                        for g in range(G):
                            gts = {}
                            for h in range(2):
                                n = int(seglen[g, h])
                                nch = n // 128
                                gt = gp.tile([128, nch * EW], bf16,
                                             tag=f"g{h}", name=f"gt{h}")
                                io = ioff[(g, h)]
                                if NOGATHER:
                                    nc.sync.dma_start(
                                        gt[:],
                                        full[0:128 * nch, :].rearrange(
                                            "(p c) d -> p (c d)", p=128))
                                else:
                                    n1 = (n // 256) * 128
                                    for (o0, nn) in ((0, n1), (n1, n - n1)):
                                        if nn == 0:
                                            continue
                                        nc.gpsimd.dma_gather(
                                            out_ap=gt[:, o0 * EW // 128:
                                                      (o0 + nn) * EW // 128]
                                            .rearrange(
                                                "p (c d) -> p c d", d=EW),
                                            in_ap=fv[:, h * EW:(h + 1) * EW],
                                            idxs_ap=ix_s[:, io + o0 // 16:
                                                         io + (o0 + nn) // 16],
                                            num_idxs=nn, num_idxs_reg=nn,
                                            elem_size=EW, elem_step=2 * EW,
                                            queue_num=qctr[0] % 4,
                                            single_packet=False,
                                        )
                                        qctr[0] += 1
                                gts[h] = gt
                            for wl in range(WPG):
                                w = g * WPG + wl
                                ps = aps.tile([128, DH], f32, tag="agg",
                                              name="agg")
                                chunks = (
                                    [(0, k) for k in range(int(tgt[w, 0]) // 128)]
                                    + [(1, k) for k in range(int(tgt[w, 1]) // 128)]
                                )
                                for j, (h, k) in enumerate(chunks):
                                    tcol = int(wcol[w, h] - coff[(g, h)]) + k
                                    dcol = int(wcol[w, h]) + k
                                    bc = bp.tile([128, 128], bf16, tag="bc",
                                                 name="bc")
                                    nc.scalar.activation(
                                        bc[:], gi_s[:], func=AF_Relu,
                                        scale=0.0,
                                        bias=dvl_s[:, dcol:dcol + 1])
                                    S = sp.tile([128, 128], bf16, tag="S",
                                                name="S")
                                    nc.vector.tensor_tensor(
                                        out=S[:], in0=bc[:], in1=gi_s[:],
                                        op=mybir.AluOpType.is_equal,
                                    )
                                    base = tcol * EW
                                    nc.tensor.matmul(
                                        ps[:, :ncols],
                                        lhsT=S[:],
                                        rhs=gts[h][:, base:base + ncols],
                                        start=(j == 0), stop=False,
                                    )
                                    nc.tensor.matmul(
                                        ps[:, :ncols],
                                        lhsT=S[:],
                                        rhs=gts[h][:, base + DH:base + DH + ncols],
                                        start=False,
                                        stop=(j == len(chunks) - 1),
                                    )
                                flush_fn(w, ps[:, :ncols])

                if STAGE == 3:
                    with tc.tile_pool(name="dbg", bufs=2) as dbg:
                        def flush_dbg(w, agg):
                            d2 = dbg.tile([128, DOUT], f32, name="d2")
                            nc.vector.tensor_copy(d2[:], agg[:, :DOUT])
                            nc.sync.dma_start(
                                out_d.ap()[w * 128:(w + 1) * 128, :], d2[:])
                        aggregate(t1_full[:], DH, flush_dbg)
                    nc.compile()
                    return nc

                # ---- layer-1 flush: h=relu(dinv*agg+b1); T2'=(dinv.*h)@W2
                with (
                    tc.tile_pool(name="fl_sb", bufs=3) as fsb,
                    tc.tile_pool(name="fl_ps", bufs=2, space="PSUM") as fps,
                ):
                    def flush1(w, agg):
                        v = fsb.tile([128, DH], f32, tag="v", name="v")
                        nc.vector.tensor_scalar_mul(
                            v[:], agg, dv_s[:, w:w + 1])
                        v2 = fsb.tile([128, DH], f32, tag="v2", name="v2")
                        nc.vector.tensor_add(out=v2[:], in0=v[:], in1=b1_s[:])
                        h2 = fsb.tile([128, DH], f32, tag="h2", name="h2")
                        # dinv*relu(x) == relu(dinv*x) since dinv>0
                        nc.scalar.activation(h2[:], v2[:], func=AF.Relu,
                                             scale=dv_s[:, w:w + 1])
                        htp = fps.tile([DH, 128], f32, tag="htp", name="htp")
                        nc.tensor.transpose(htp[:], h2[:], id_s[:])
                        hts = fsb.tile([DH, 128], f32, tag="hts", name="hts")
                        nc.vector.tensor_copy(hts[:], htp[:])
                        t2p = fps.tile([128, DOUT], f32, tag="t2p", name="t2p")
                        nc.tensor.matmul(t2p[:], lhsT=hts[:], rhs=w2_s[:],
                                         start=True, stop=True)
                        t2f = fsb.tile([128, DOUT], f32, tag="t2f32",
                                       name="t2f32")
                        nc.vector.tensor_copy(t2f[:], t2p[:])
                        # layout: hi at [0,DOUT), lo at [DH,DH+DOUT)
                        t2sb = fsb.tile([128, 2 * DH], bf16, tag="t2sb",
                                        name="t2sb")
                        nc.vector.memset(t2sb[:, DOUT:DH], 0.0)
                        nc.vector.memset(t2sb[:, DH + DOUT:], 0.0)
                        nc.vector.tensor_copy(t2sb[:, :DOUT], t2f[:])
                        t2ib = fsb.tile([128, DOUT], f32, tag="t2ib",
                                        name="t2ib")
                        nc.vector.tensor_copy(t2ib[:], t2sb[:, :DOUT])
                        nc.vector.tensor_tensor(
                            out=t2sb[:, DH:DH + DOUT], in0=t2f[:],
                            in1=t2ib[:], op=mybir.AluOpType.subtract)
                        nc.sync.dma_start(
                            t2_shard[w * 128:(w + 1) * 128, :], t2sb[:])

                    aggregate(t1_full[:], DH, flush1)

                    # ---- phase 4: AllGather layer-2 table
                    nc.gpsimd.collective_compute(
                        "AllGather", mybir.AluOpType.bypass, replica_groups=rg,
                        ins=[t2_shard[:]], outs=[t2_full[:]],
                    )

                    if STAGE == 4:
                        with tc.tile_pool(name="dbg", bufs=2) as dbg:
                            for w in range(WPC):
                                d = dbg.tile([128, DOUT], f32, name="d")
                                nc.sync.dma_start(
                                    d[:],
                                    t2_full[w * 128:(w + 1) * 128, :DOUT])
                                d2 = dbg.tile([128, DOUT], f32, name="d2")
                                nc.vector.tensor_copy(d2[:], d[:])
                                nc.sync.dma_start(
                                    out_d.ap()[w * 128:(w + 1) * 128, :],
                                    d2[:])
                        nc.compile()
                        return nc

                    # ---- layer-2 flush: h2 + attention gate -> out
                    def flush2(w, agg):
                        v = fsb.tile([128, DOUT], f32, tag="f2v", name="f2v")
                        nc.vector.tensor_scalar_mul(
                            v[:], agg, dv_s[:, w:w + 1])
                        v2 = fsb.tile([128, DOUT], f32, tag="f2v2",
                                      name="f2v2")
                        nc.vector.tensor_add(out=v2[:], in0=v[:], in1=b2_s[:])
                        hh = fsb.tile([128, DOUT], f32, tag="f2h", name="f2h")
                        nc.scalar.activation(hh[:], v2[:], func=AF.Relu)
                        a = fsb.tile([128, DOUT], f32, tag="f2a", name="f2a")
                        nc.vector.tensor_mul(out=a[:], in0=hh[:], in1=aw_s[:])
                        ar = fsb.tile([128, 1], f32, tag="f2ar", name="f2ar")
                        nc.vector.tensor_reduce(
                            ar[:], a[:], axis=mybir.AxisListType.X,
                            op=mybir.AluOpType.add)
                        at = fsb.tile([128, 1], f32, tag="f2at", name="f2at")
                        nc.scalar.activation(at[:], ar[:], func=AF.Sigmoid,
                                             bias=ab_s[:, :1])
                        o = fsb.tile([128, DOUT], f32, tag="f2o", name="f2o")
                        nc.vector.tensor_scalar_mul(o[:], hh[:], at[:])
                        nc.sync.dma_start(
                            out_d.ap()[w * 128:(w + 1) * 128, :], o[:])

                    aggregate(t2_full[:], DOUT, flush2)

    nc.compile()
    return nc


# ---------------------------------------------------------------------------
# entry point
# ---------------------------------------------------------------------------


def _make_in_maps(cfg, host, W1, b1, W2, b2, attn_w, attn_b):
    import ml_dtypes
    NC = cfg.NC
    bf16 = ml_dtypes.bfloat16
    ident = np.eye(128, dtype=np.float32)
    giota = np.tile(np.arange(2, 130, dtype=np.float32), (128, 1)).astype(bf16)
    in_maps = []
    for c in range(NC):
        in_maps.append({
            "x_sh": host["x_sh"][c],
            "w1": np.asarray(W1, np.float32),
            "w2": np.asarray(W2, np.float32),
            "b1rep": np.tile(np.asarray(b1, np.float32), (128, 1)),
            "b2rep": np.tile(np.asarray(b2, np.float32), (128, 1)),
            "awrep": np.tile(np.asarray(attn_w, np.float32).reshape(1, -1),
                             (128, 1)),
            "abcol": np.full((128, 1),
                             np.asarray(attn_b, np.float32).reshape(-1)[0],
                             np.float32),
            "dinv_slot": host["dinv_slot"][c],
            "ident": ident,
            "giota": giota,
            "idx_all": host["idx_all"][c],
            "dval_all": host["dval_all"][c],
        })
    return in_maps


def run(x, edge_index, W1, b1, W2, b2, attn_w, attn_b, cfg=None,
        backend="hw", trace=False):
    cfg = cfg or FULL
    plan, host = prep(x, edge_index, cfg)
    nc = build(cfg, plan)
    in_maps = _make_in_maps(cfg, host, W1, b1, W2, b2, attn_w, attn_b)

    if backend == "sim":
        from concourse.bass_interp import MultiCoreSim
        sim = MultiCoreSim(nc, num_cores=cfg.NC, trace=False)
        for c, core in enumerate(sim.cores.values()):
            for name, arr in in_maps[c].items():
                core.tensor(name)[:] = arr
        sim.simulate()
        outs = [core.tensor("out_sh").copy() for core in sim.cores.values()]
        exec_ns = None
    else:
        from concourse import bass_utils
        from concourse.bass_interp import get_hw_module
        old = nc.m
        nc.m = get_hw_module(nc.m)
        try:
            res = bass_utils.run_bass_kernel_spmd(
                nc, in_maps, core_ids=list(range(cfg.NC)), trace=trace)
        finally:
            nc.m = old
        outs = [res.results[c]["out_sh"] for c in range(cfg.NC)]
        exec_ns = res.exec_time_ns

    full = np.concatenate(outs, axis=0)  # [TOT, DOUT] in slot order
    out = full[host["pos"]]              # unpermute -> [N, DOUT]
    return np.ascontiguousarray(out), exec_ns


def kernel(x, edge_index, W1, b1, W2, b2, attn_w, attn_b):
    out, _ = run(x, edge_index, W1, b1, W2, b2, attn_w, attn_b,
                 cfg=FULL, backend="hw", trace=False)
    return out
